# revision 25
# baseline (speedup 1.0000x reference)
"""Bass/Tile kernel for EpisodeMultiheadAttentionBlock on TRN2.

Per-core: 2 batch elements (data-parallel over B=16 across 8 cores).

Layout: activations feature-major [feature parts, token free]; V token-major.
Projections/GRU matmuls float32r (1 cyc/row at free>=256); attention island
(scores, exp/pT, V, rowsum) in bf16 for 2x DVE + half DMA. Softmax without
max-subtraction; multiplicative 0/1 mask applied as ONE coarse op per head
group; 1/rowsum via ones-matmul broadcast; attn@V consumes unnormalized pT
(normalized on the [64,QL] eviction); attn_w accumulated in f32 from a
Pool-engine coarse product. Attention groups are software-pipelined
(scores/exp of group g+1 issued before the reduction half of group g).
"""

import numpy as np

import concourse.bass as bass
import concourse.mybir as mybir
import concourse.tile as tile
from concourse import bacc

F32 = mybir.dt.float32
F32R = mybir.dt.float32r
BF16 = mybir.dt.bfloat16
AF = mybir.ActivationFunctionType

B_PER_CORE = 2
L = 1024
E = 1024
H = 16
QL = 256
NCH = 8

SINGLES = ["wq", "wk", "wv", "wo"]
GRUCATS = ["wrC", "wzC", "wgC"]


def build_kernel(num_devices=8):
    nc = bacc.Bacc("TRN2", target_bir_lowering=False, debug=False,
                   num_devices=num_devices)

    kinT_d = nc.dram_tensor("kinT", [B_PER_CORE, E, L], BF16, kind="ExternalInput")
    keyT_d = nc.dram_tensor("keyT", [B_PER_CORE, E, L], F32R, kind="ExternalInput")
    maskT_d = nc.dram_tensor("maskT", [B_PER_CORE, L, QL], BF16, kind="ExternalInput")
    ones_d = nc.dram_tensor("ones", [128, 128], BF16, kind="ExternalInput")
    w_d = {n: nc.dram_tensor(n + "T", [E, E],
                             BF16 if n in ("wq", "wk") else F32R,
                             kind="ExternalInput")
           for n in SINGLES}
    for n in GRUCATS:
        w_d[n] = nc.dram_tensor(n, [2 * E, E], F32R, kind="ExternalInput")
    b_d = {n: nc.dram_tensor(n, [E], F32, kind="ExternalInput")
           for n in ["bq", "bk", "bo", "brz", "bzz", "bgg"]}
    bv_d = nc.dram_tensor("bv", [1, E], F32, kind="ExternalInput")

    kts_d = nc.dram_tensor("kts", [B_PER_CORE, NCH, 128, L], BF16)
    outT_d = nc.dram_tensor("outT", [B_PER_CORE, E, QL], F32, kind="ExternalOutput")
    attnwT_d = nc.dram_tensor("attnwT", [B_PER_CORE, L, QL], F32,
                              kind="ExternalOutput")

    with tile.TileContext(nc) as tc, nc.allow_low_precision(
            reason="bf16 attention island feeds f32 PSUM; f32 accumulators"):
        _body(nc, tc, kinT_d, keyT_d, maskT_d, ones_d, w_d, b_d, bv_d,
              kts_d, outT_d, attnwT_d)
    nc.compile()
    return nc


def _bcast_mid(ap, n, pos):
    """Insert a step-0 dim of extent n at position pos of an AP."""
    lst = [list(p) for p in ap.ap]
    return bass.AP(tensor=ap.tensor, offset=ap.offset,
                   ap=lst[:pos] + [[0, n]] + lst[pos:])


def _body(nc, tc, kinT_d, keyT_d, maskT_d, ones_d, w_d, b_d, bv_d,
          kts_d, outT_d, attnwT_d):
    from contextlib import ExitStack
    ctx = ExitStack()
    with ctx:
        consts = ctx.enter_context(tc.tile_pool(name="consts", bufs=1))
        wpool = ctx.enter_context(tc.tile_pool(name="wpool", bufs=4))
        big = ctx.enter_context(tc.tile_pool(name="big", bufs=1))
        med = ctx.enter_context(tc.tile_pool(name="med", bufs=1))
        small = ctx.enter_context(tc.tile_pool(name="small", bufs=2))
        ps_proj = ctx.enter_context(
            tc.tile_pool(name="ps_proj", bufs=5, space="PSUM"))
        ps_half = ctx.enter_context(
            tc.tile_pool(name="ps_half", bufs=3, space="PSUM"))

        ones_t = consts.tile([128, 128], BF16, tag="ones")
        nc.sync.dma_start(ones_t[:], ones_d[:])
        bias_sb = {}
        for n, d in b_d.items():
            t = consts.tile([128, NCH], F32, tag="b_" + n)
            nc.sync.dma_start(t[:], d.rearrange("(c p) -> p c", p=128))
            bias_sb[n] = t
        bvb = consts.tile([128, E], F32, tag="bvb")
        bv_ap = bv_d[:, :]
        nc.gpsimd.dma_start(
            out=bvb[:],
            in_=bass.AP(tensor=bv_ap.tensor, offset=bv_ap.offset,
                        ap=[[0, 128], [1, E]]))

        # persistent double-buffered query staging; zero halves set once
        qzd = consts.tile([128, 2, 2, QL], BF16, tag="qzd")
        nc.gpsimd.memset(qzd[64:128, 0, 0, :], 0.0)
        nc.gpsimd.memset(qzd[0:64, 0, 1, :], 0.0)
        nc.gpsimd.memset(qzd[64:128, 1, 0, :], 0.0)
        nc.gpsimd.memset(qzd[0:64, 1, 1, :], 0.0)

        def load_wq(name, qo, dt=F32R):
            """Quarter qo (out-cols qo*256..+256) of a single [E,E] weight."""
            t = wpool.tile([128, NCH, 256], dt, tag="w", name=f"{name}_{qo}")
            nc.sync.dma_start(
                t[:],
                w_d[name][:, qo * 256:(qo + 1) * 256]
                .rearrange("(c p) o -> p c o", p=128))
            return t

        def load_w8(name, mo):
            """Eighth mo (out-cols mo*128..+128) of a [2E,E] GRU cat weight."""
            t = wpool.tile([128, 2 * NCH, 128], F32R, tag="w",
                           name=f"{name}_{mo}")
            nc.sync.dma_start(
                t[:],
                w_d[name][:, mo * 128:(mo + 1) * 128]
                .rearrange("(c p) o -> p c o", p=128))
            return t

        def load_oriq(b, phase):
            t = med.tile([128, NCH, QL], F32R, tag="oriq", bufs=2,
                         name=f"oriq_{phase}_{b}")
            nc.sync.dma_start(
                t[:], keyT_d[b][:, L - QL:].rearrange("(c p) t -> p c t", p=128))
            return t

        outT = [None] * B_PER_CORE
        aoutT = [None] * B_PER_CORE

        # ================= PHASE A (per batch) =================
        for b in range(B_PER_CORE):
            # weights first on the sync queue so the first matmul starts early
            wq_q = [load_wq("wq", qo, BF16) for qo in range(4)]
            kin = big.tile([128, NCH, L], BF16, tag="kin_vkm", name=f"kin{b}")
            kin_re = kinT_d[b].rearrange("(c p) t -> p c t", p=128)
            # q-projection token slice first so PE can start ASAP
            nc.sync.dma_start(kin[:, :, L - QL:], kin_re[:, :, L - QL:])
            nc.gpsimd.dma_start(kin[:, :, 0:512], kin_re[:, :, 0:512])
            nc.gpsimd.dma_start(kin[:, :, 512:L - QL], kin_re[:, :, 512:L - QL])
            maskt = big.tile([128, NCH, QL], BF16, tag="mask", name=f"mask{b}")
            nc.scalar.dma_start(maskt[:], maskT_d[b].rearrange("(c p) t -> p c t", p=128))
            qT = med.tile([128, NCH, QL], F32R, tag="qrh", bufs=2, name=f"qT{b}")
            for mo in range(NCH):
                p = ps_half.tile([128, QL], F32, tag="half")
                wt = wq_q[mo // 2]
                for ci in range(NCH):
                    nc.tensor.matmul(
                        p[:], wt[:, ci, (mo % 2) * 128:(mo % 2) * 128 + 128],
                        kin[:, ci, L - QL:],
                        start=(ci == 0), stop=(ci == NCH - 1))
                nc.scalar.activation(qT[:, mo, :], p[:], AF.Identity,
                                     bias=bias_sb["bq"][:, mo:mo + 1])

            # ---- k projection (streamed to DRAM scratch as bf16) ----
            wk_q = [load_wq("wk", qo, BF16) for qo in range(4)]
            for mo in range(NCH):
                wt = wk_q[mo // 2]
                for n in range(2):
                    p = ps_proj.tile([128, 512], F32, tag="proj")
                    for ci in range(NCH):
                        nc.tensor.matmul(
                            p[:], wt[:, ci, (mo % 2) * 128:(mo % 2) * 128 + 128],
                            kin[:, ci, n * 512:(n + 1) * 512],
                            start=(ci == 0), stop=(ci == NCH - 1))
                    kt = small.tile([128, 512], BF16, tag="ktmp", bufs=2)
                    nc.scalar.activation(kt[:], p[:], AF.Identity,
                                         bias=bias_sb["bk"][:, mo:mo + 1])
                    nc.gpsimd.dma_start(kts_d[b, mo, :, n * 512:(n + 1) * 512],
                                        kt[:])

            # ---- v projection (token-major, bf16); kin slot reused ----
            wv_q = [load_wq("wv", qo) for qo in range(4)]
            vkm = big.tile([128, NCH, E], BF16, tag="kin_vkm", name=f"vkm{b}")
            for kc in range(NCH):
                keyc = small.tile([128, NCH, 128], F32R, tag="keyc", bufs=1)
                nc.gpsimd.dma_start(
                    keyc[:],
                    keyT_d[b][:, kc * 128:(kc + 1) * 128]
                    .rearrange("(c p) t -> p c t", p=128))
                for q4 in range(4):
                    p = ps_half.tile([128, 256], F32, tag="half")
                    for ci in range(NCH):
                        nc.tensor.matmul(
                            p[:], keyc[:, ci, :], wv_q[q4][:, ci, :],
                            start=(ci == 0), stop=(ci == NCH - 1))
                    nc.vector.tensor_add(
                        vkm[:, kc, q4 * 256:(q4 + 1) * 256], p[:],
                        bvb[:, q4 * 256:(q4 + 1) * 256])

            # ---- attention: software-pipelined over 2-head groups ----
            awA = med.tile([128, NCH, QL], F32, tag="awA", name=f"awA{b}")
            aoutT[b] = med.tile([128, NCH, QL], F32R, tag="aoutT",
                                bufs=2, name=f"aoutT{b}")
            pTs = [None] * NCH

            def front(g):
                ktd = med.tile([128, L], BF16, tag="ktd", bufs=2,
                               name=f"ktd{b}_{g}")
                nc.scalar.dma_start(ktd[:], kts_d[b, g])
                s = g % 2
                nc.vector.tensor_copy(qzd[0:64, s, 0, :],
                                      qT[0:64, g, :].bitcast(F32))
                nc.vector.tensor_copy(qzd[64:128, s, 1, :],
                                      qT[64:128, g, :].bitcast(F32))
                pT = med.tile([128, NCH, 2, QL], BF16, tag="pT", bufs=2,
                              name=f"pT{b}_{g}")
                pTs[g] = pT
                for kc in range(NCH):
                    ps = ps_proj.tile([128, 2, QL], F32, tag="proj")
                    nc.tensor.matmul(
                        ps[:].rearrange("p a b -> p (a b)"),
                        ktd[:, kc * 128:(kc + 1) * 128],
                        qzd[:, s].rearrange("p a b -> p (a b)"),
                        start=True, stop=True)
                    nc.scalar.activation(pT[:, kc], ps[:], AF.Exp, scale=0.125)
                # coarse mask over all kc / both halves (bf16, Pool engine)
                nc.gpsimd.tensor_mul(pT[:], pT[:],
                                     _bcast_mid(maskt[:], 2, 2))

            def back(g):
                pT = pTs[g]
                # rowsum of masked exp, broadcast via ones-matmul
                rp = ps_proj.tile([128, 2, QL], F32, tag="proj")
                for kc in range(NCH):
                    nc.tensor.matmul(rp[:].rearrange("p a b -> p (a b)"),
                                     ones_t[:],
                                     pT[:, kc].rearrange("p a b -> p (a b)"),
                                     start=(kc == 0), stop=(kc == NCH - 1))
                r1f = small.tile([1, 2, QL], F32, tag="r1f", bufs=1)
                nc.vector.reciprocal_approx_fast(r1f[:], rp[0:1, :, :])
                r1 = small.tile([1, 2, QL], BF16, tag="r1", bufs=2)
                nc.vector.tensor_copy(r1[:], r1f[:])

                # attn @ V per head (M=64) on unnormalized masked pT
                pav = [ps_half.tile([64, QL], F32, tag="half",
                                    name=f"pav{hi}") for hi in range(2)]
                for kc in range(NCH):
                    for hi in range(2):
                        h = 2 * g + hi
                        nc.tensor.matmul(
                            pav[hi][:, :],
                            vkm[:, kc, h * 64:(h + 1) * 64],
                            pT[:, kc, hi, :],
                            start=(kc == 0), stop=(kc == NCH - 1))

                # broadcast reciprocal to all partitions
                rbp = ps_proj.tile([128, 2, QL], F32, tag="proj")
                nc.tensor.matmul(rbp[:].rearrange("p a b -> p (a b)"),
                                 ones_t[0:1, :],
                                 r1[:].rearrange("p a b -> p (a b)"),
                                 start=True, stop=True)
                rb16 = small.tile([128, 2, QL], BF16, tag="rb16", bufs=2)
                nc.scalar.activation(rb16[:], rbp[:], AF.Copy)
                recipb = small.tile([64, 2, QL], F32, tag="recipb", bufs=2)
                nc.vector.tensor_copy(recipb[:], rbp[0:64])

                # normalize attn@V during eviction; odd head partition-shifted
                nc.vector.tensor_mul(aoutT[b][0:64, g, :], pav[0][:, :],
                                     recipb[:, 0, :])
                sh = small.tile([64, QL], F32R, tag="btmp")
                nc.vector.tensor_mul(sh[:, :], pav[1][:, :],
                                     recipb[:, 1, :])
                nc.sync.dma_start(aoutT[b][64:128, g, :], sh[:, :])

                # attn_w: normalize pT in place (all-bf16 2x; attn@V already
                # consumed the unnormalized values), then f32 accumulate
                nc.vector.tensor_mul(pT[:], pT[:],
                                     _bcast_mid(rb16[:], NCH, 1))
                if g == 0:
                    nc.vector.tensor_copy(awA[:], pT[:, :, 0, :])
                else:
                    nc.vector.tensor_add(awA[:], awA[:], pT[:, :, 0, :])
                nc.vector.tensor_add(awA[:], awA[:], pT[:, :, 1, :])

            for g in range(NCH):
                front(g)
                if g > 0:
                    back(g - 1)
            back(NCH - 1)

            nc.gpsimd.tensor_scalar_mul(awA[:], awA[:], 1.0 / H)
            nc.sync.dma_start(
                attnwT_d[b].rearrange("(c p) t -> p c t", p=128), awA[:])

            # ---- out projection + relu + residual (f32r) ----
            wo_q = [load_wq("wo", qo) for qo in range(4)]
            oriq_a = load_oriq(b, "a")
            outT[b] = med.tile([128, NCH, QL], F32R, tag="outT",
                               bufs=B_PER_CORE, name=f"outT{b}")
            for mo in range(NCH):
                p = ps_half.tile([128, QL], F32, tag="half")
                wt = wo_q[mo // 2]
                for ci in range(NCH):
                    nc.tensor.matmul(
                        p[:], wt[:, ci, (mo % 2) * 128:(mo % 2) * 128 + 128],
                        aoutT[b][:, ci, :], start=(ci == 0), stop=(ci == NCH - 1))
                t = small.tile([128, QL], F32, tag="btmp")
                nc.scalar.activation(t[:], p[:], AF.Relu,
                                     bias=bias_sb["bo"][:, mo:mo + 1])
                nc.vector.tensor_add(outT[b][:, mo, :], t[:],
                                     oriq_a[:, mo, :].bitcast(F32))

        # ================= PHASE B: gated GRU (both batches) =================
        rqT = [None] * B_PER_CORE
        for b in range(B_PER_CORE):
            rqT[b] = med.tile([128, NCH, QL], F32R, tag="qrh", bufs=2, name=f"rqT{b}")
        hT = [None] * B_PER_CORE

        for stage, (wname, bias, func) in enumerate(
                [("wrC", "brz", AF.Relu), ("wgC", "bgg", AF.Tanh),
                 ("wzC", "bzz", AF.Relu)]):
            oriq_g = ([load_oriq(b, f"g{stage}") for b in range(B_PER_CORE)]
                      if stage != 1 else [None] * B_PER_CORE)
            if stage == 1:
                for b in range(B_PER_CORE):
                    hT[b] = med.tile([128, NCH, QL], F32, tag="aoutT",
                                     bufs=2, name=f"hT{b}")
            for mo in range(NCH):
                w8 = load_w8(wname, mo)
                for b in range(B_PER_CORE):
                    p = ps_half.tile([128, QL], F32, tag="half")
                    xside = oriq_g[b] if stage != 1 else rqT[b]
                    for ci in range(2 * NCH):
                        rhs = (xside[:, ci, :] if ci < NCH
                               else outT[b][:, ci - NCH, :])
                        nc.tensor.matmul(
                            p[:], w8[:, ci, :], rhs,
                            start=(ci == 0), stop=(ci == 2 * NCH - 1))
                    if stage == 0:   # r -> rq
                        t = small.tile([128, QL], F32, tag="btmp")
                        nc.scalar.activation(t[:], p[:], func,
                                             bias=bias_sb[bias][:, mo:mo + 1])
                        nc.vector.tensor_mul(rqT[b][:, mo, :], t[:],
                                             oriq_g[b][:, mo, :].bitcast(F32))
                    elif stage == 1:  # h
                        nc.scalar.activation(hT[b][:, mo, :], p[:], func,
                                             bias=bias_sb[bias][:, mo:mo + 1])
                    else:            # z + final blend + store
                        zt = small.tile([128, QL], F32, tag="btmp")
                        nc.scalar.activation(zt[:], p[:], func,
                                             bias=bias_sb[bias][:, mo:mo + 1])
                        d = small.tile([128, QL], F32, tag="btmp")
                        nc.vector.tensor_sub(d[:], hT[b][:, mo, :],
                                             oriq_g[b][:, mo, :].bitcast(F32))
                        nc.vector.tensor_mul(d[:], d[:], zt[:])
                        fin = small.tile([128, QL], F32, tag="btmp")
                        nc.vector.tensor_add(fin[:], d[:],
                                             oriq_g[b][:, mo, :].bitcast(F32))
                        nc.sync.dma_start(
                            outT_d[b][mo * 128:(mo + 1) * 128, :], fin[:])


def prep_inputs_core(core, key, pe, key_index, key_padding_mask,
                     in_proj_w, in_proj_b, out_w, out_b, gw, gb):
    b0 = core * B_PER_CORE
    sl = slice(b0, b0 + B_PER_CORE)
    import ml_dtypes as _mld
    keyc = np.asarray(key[sl], np.float32)
    kin = keyc + np.asarray(pe[sl], np.float32)
    kinT = np.ascontiguousarray(kin.transpose(0, 2, 1)).astype(_mld.bfloat16)
    keyT = np.ascontiguousarray(keyc.transpose(0, 2, 1))

    ki = np.asarray(key_index[sl])
    pad = np.asarray(key_padding_mask[sl])
    qi = ki[:, L - QL:]
    ri = ki[:, :L - QL]
    import ml_dtypes
    allowed = np.zeros((B_PER_CORE, L, QL), np.float32)
    allowed[:, :L - QL, :] = ((ri[:, :, None] < qi[:, None, :])
                              & ~pad[:, :L - QL, None])
    allowed[:, L - QL:, :] = np.eye(QL, dtype=np.float32)[None]

    w32 = lambda x: np.asarray(x, np.float32)
    im = {
        "kinT": kinT, "keyT": keyT,
        "maskT": allowed.astype(ml_dtypes.bfloat16),
        "ones": np.ones((128, 128), ml_dtypes.bfloat16),
        "bv": w32(in_proj_b[2 * E:]).reshape(1, E),
        "bq": w32(in_proj_b[:E]),
        "bk": w32(in_proj_b[E:2 * E]),
        "bo": w32(out_b),
        "brz": w32(gb["bxr"] + gb["byr"]),
        "bzz": w32(gb["bxz"] + gb["byz"]),
        "bgg": w32(gb["bxg"] + gb["byg"]),
        "wqT": np.ascontiguousarray(w32(in_proj_w[:E]).T).astype(_mld.bfloat16),
        "wkT": np.ascontiguousarray(w32(in_proj_w[E:2 * E]).T).astype(_mld.bfloat16),
        "wvT": np.ascontiguousarray(w32(in_proj_w[2 * E:]).T),
        "woT": np.ascontiguousarray(w32(out_w).T),
        "wrC": np.ascontiguousarray(
            np.concatenate([w32(gw["wxr"]).T, w32(gw["wyr"]).T], 0)),
        "wzC": np.ascontiguousarray(
            np.concatenate([w32(gw["wxz"]).T, w32(gw["wyz"]).T], 0)),
        "wgC": np.ascontiguousarray(
            np.concatenate([w32(gw["wxg"]).T, w32(gw["wyg"]).T], 0)),
    }
    return im


def postprocess(results):
    outs, aws = [], []
    for r in results:
        outs.append(r["outT"].transpose(0, 2, 1))
        aws.append(r["attnwT"].transpose(0, 2, 1))
    return (np.concatenate(outs, 0), np.concatenate(aws, 0))


_NC_CACHE = {}


def kernel(key, pe, key_index, key_padding_mask, query_length,
           in_proj_w, in_proj_b, out_w, out_b,
           wxr, bxr, wyr, byr, wxz, bxz, wyz, byz, wxg, bxg, wyg, byg):
    """Full-input entry point: shard B=16 across 8 NeuronCores, run, gather."""
    from concourse.bass_utils import run_bass_kernel_spmd

    key = np.asarray(key)
    assert int(query_length) == QL and key.shape == (16, L, E)
    if "nc" not in _NC_CACHE:
        _NC_CACHE["nc"] = build_kernel(num_devices=8)
    nc = _NC_CACHE["nc"]

    gw = {"wxr": wxr, "wyr": wyr, "wxz": wxz, "wyz": wyz,
          "wxg": wxg, "wyg": wyg}
    gb = {"bxr": bxr, "byr": byr, "bxz": bxz, "byz": byz,
          "bxg": bxg, "byg": byg}
    in_maps = [prep_inputs_core(c, key, pe, key_index, key_padding_mask,
                                in_proj_w, in_proj_b, out_w, out_b, gw, gb)
               for c in range(8)]
    res = run_bass_kernel_spmd(nc, in_maps, core_ids=list(range(8)))
    out, attn_w = postprocess(res.results)
    return out.astype(np.float32), attn_w.astype(np.float32)


# revision 26
# speedup vs baseline: 1.0735x; 1.0735x over previous
"""Bass/Tile kernel for EpisodeMultiheadAttentionBlock on TRN2.

Per-core: 2 batch elements (data-parallel over B=16 across 8 cores).

Layout: activations feature-major [feature parts, token free]; V token-major.
Projections/GRU matmuls float32r (1 cyc/row at free>=256); attention island
(scores, exp/pT, V, rowsum) in bf16 for 2x DVE + half DMA. Softmax without
max-subtraction; multiplicative 0/1 mask applied as ONE coarse op per head
group; 1/rowsum via ones-matmul broadcast; attn@V consumes unnormalized pT
(normalized on the [64,QL] eviction); attn_w accumulated in f32 from a
Pool-engine coarse product. Attention groups are software-pipelined
(scores/exp of group g+1 issued before the reduction half of group g).
"""

import numpy as np

import concourse.bass as bass
import concourse.mybir as mybir
import concourse.tile as tile
from concourse import bacc

F32 = mybir.dt.float32
F32R = mybir.dt.float32r
BF16 = mybir.dt.bfloat16
AF = mybir.ActivationFunctionType

B_PER_CORE = 2
L = 1024
E = 1024
H = 16
QL = 256
NCH = 8

SINGLES = ["wq", "wk", "wv", "wo"]
GRUCATS = ["wrC", "wzC", "wgC"]


def build_kernel(num_devices=8):
    nc = bacc.Bacc("TRN2", target_bir_lowering=False, debug=False,
                   num_devices=num_devices)

    kinT_d = nc.dram_tensor("kinT", [B_PER_CORE, E, L], BF16, kind="ExternalInput")
    keyT_d = nc.dram_tensor("keyT", [B_PER_CORE, E, L], F32R, kind="ExternalInput")
    maskT_d = nc.dram_tensor("maskT", [B_PER_CORE, L, QL], BF16, kind="ExternalInput")
    ones_d = nc.dram_tensor("ones", [128, 128], BF16, kind="ExternalInput")
    w_d = {n: nc.dram_tensor(n + "T", [E, E],
                             BF16 if n in ("wq", "wk") else F32R,
                             kind="ExternalInput")
           for n in SINGLES}
    for n in GRUCATS:
        w_d[n] = nc.dram_tensor(n, [2 * E, E], F32R, kind="ExternalInput")
    b_d = {n: nc.dram_tensor(n, [E], F32, kind="ExternalInput")
           for n in ["bq", "bk", "bo", "brz", "bzz", "bgg"]}
    bv_d = nc.dram_tensor("bv", [1, E], F32, kind="ExternalInput")

    kts_d = nc.dram_tensor("kts", [B_PER_CORE, NCH, 128, L], BF16)
    outT_d = nc.dram_tensor("outT", [B_PER_CORE, E, QL], F32, kind="ExternalOutput")
    attnwT_d = nc.dram_tensor("attnwT", [B_PER_CORE, L, QL], F32,
                              kind="ExternalOutput")

    with tile.TileContext(nc) as tc, nc.allow_low_precision(
            reason="bf16 attention island feeds f32 PSUM; f32 accumulators"):
        _body(nc, tc, kinT_d, keyT_d, maskT_d, ones_d, w_d, b_d, bv_d,
              kts_d, outT_d, attnwT_d)
    nc.compile()
    return nc


def _bcast_mid(ap, n, pos):
    """Insert a step-0 dim of extent n at position pos of an AP."""
    lst = [list(p) for p in ap.ap]
    return bass.AP(tensor=ap.tensor, offset=ap.offset,
                   ap=lst[:pos] + [[0, n]] + lst[pos:])


def _body(nc, tc, kinT_d, keyT_d, maskT_d, ones_d, w_d, b_d, bv_d,
          kts_d, outT_d, attnwT_d):
    from contextlib import ExitStack
    ctx = ExitStack()
    with ctx:
        consts = ctx.enter_context(tc.tile_pool(name="consts", bufs=1))
        wpool = ctx.enter_context(tc.tile_pool(name="wpool", bufs=4))
        big = ctx.enter_context(tc.tile_pool(name="big", bufs=1))
        med = ctx.enter_context(tc.tile_pool(name="med", bufs=1))
        small = ctx.enter_context(tc.tile_pool(name="small", bufs=2))
        ps_proj = ctx.enter_context(
            tc.tile_pool(name="ps_proj", bufs=5, space="PSUM"))
        ps_half = ctx.enter_context(
            tc.tile_pool(name="ps_half", bufs=3, space="PSUM"))

        ones_t = consts.tile([128, 128], BF16, tag="ones")
        nc.sync.dma_start(ones_t[:], ones_d[:])
        bias_sb = {}
        for n, d in b_d.items():
            t = consts.tile([128, NCH], F32, tag="b_" + n)
            nc.sync.dma_start(t[:], d.rearrange("(c p) -> p c", p=128))
            bias_sb[n] = t
        bvb = consts.tile([128, E], F32, tag="bvb")
        bv_ap = bv_d[:, :]
        nc.gpsimd.dma_start(
            out=bvb[:],
            in_=bass.AP(tensor=bv_ap.tensor, offset=bv_ap.offset,
                        ap=[[0, 128], [1, E]]))

        # persistent double-buffered query staging; zero halves set once
        qzd = consts.tile([128, 2, 2, QL], BF16, tag="qzd")
        nc.gpsimd.memset(qzd[64:128, 0, 0, :], 0.0)
        nc.gpsimd.memset(qzd[0:64, 0, 1, :], 0.0)
        nc.gpsimd.memset(qzd[64:128, 1, 0, :], 0.0)
        nc.gpsimd.memset(qzd[0:64, 1, 1, :], 0.0)

        def load_wq(name, qo, dt=F32R):
            """Quarter qo (out-cols qo*256..+256) of a single [E,E] weight."""
            t = wpool.tile([128, NCH, 256], dt, tag="w", name=f"{name}_{qo}")
            nc.sync.dma_start(
                t[:],
                w_d[name][:, qo * 256:(qo + 1) * 256]
                .rearrange("(c p) o -> p c o", p=128))
            return t

        def load_w8(name, mo):
            """Eighth mo (out-cols mo*128..+128) of a [2E,E] GRU cat weight."""
            t = wpool.tile([128, 2 * NCH, 128], F32R, tag="w",
                           name=f"{name}_{mo}")
            nc.sync.dma_start(
                t[:],
                w_d[name][:, mo * 128:(mo + 1) * 128]
                .rearrange("(c p) o -> p c o", p=128))
            return t

        def load_oriq(b, phase):
            t = med.tile([128, NCH, QL], F32R, tag="oriq", bufs=2,
                         name=f"oriq_{phase}_{b}")
            nc.sync.dma_start(
                t[:], keyT_d[b][:, L - QL:].rearrange("(c p) t -> p c t", p=128))
            return t

        outT = [None] * B_PER_CORE
        aoutT = [None] * B_PER_CORE

        # ================= PHASE A (per batch) =================
        for b in range(B_PER_CORE):
            # weights first on the sync queue so the first matmul starts early
            wq_q = [load_wq("wq", qo, BF16) for qo in range(4)]
            kin = big.tile([128, NCH, L], BF16, tag="kin_vkm", name=f"kin{b}")
            kin_re = kinT_d[b].rearrange("(c p) t -> p c t", p=128)
            # q-projection token slice first so PE can start ASAP
            nc.sync.dma_start(kin[:, :, L - QL:], kin_re[:, :, L - QL:])
            nc.gpsimd.dma_start(kin[:, :, 0:512], kin_re[:, :, 0:512])
            nc.gpsimd.dma_start(kin[:, :, 512:L - QL], kin_re[:, :, 512:L - QL])
            maskt = big.tile([128, NCH, QL], BF16, tag="mask", name=f"mask{b}")
            nc.scalar.dma_start(maskt[:], maskT_d[b].rearrange("(c p) t -> p c t", p=128))
            qT = med.tile([128, NCH, QL], F32R, tag="qrh", bufs=2, name=f"qT{b}")
            for mo in range(NCH):
                p = ps_half.tile([128, QL], F32, tag="half")
                wt = wq_q[mo // 2]
                for ci in range(NCH):
                    nc.tensor.matmul(
                        p[:], wt[:, ci, (mo % 2) * 128:(mo % 2) * 128 + 128],
                        kin[:, ci, L - QL:],
                        start=(ci == 0), stop=(ci == NCH - 1))
                nc.scalar.activation(qT[:, mo, :], p[:], AF.Identity,
                                     bias=bias_sb["bq"][:, mo:mo + 1])

            # ---- k projection (streamed to DRAM scratch as bf16) ----
            wk_q = [load_wq("wk", qo, BF16) for qo in range(4)]
            for mo in range(NCH):
                wt = wk_q[mo // 2]
                for n in range(2):
                    p = ps_proj.tile([128, 512], F32, tag="proj")
                    for ci in range(NCH):
                        nc.tensor.matmul(
                            p[:], wt[:, ci, (mo % 2) * 128:(mo % 2) * 128 + 128],
                            kin[:, ci, n * 512:(n + 1) * 512],
                            start=(ci == 0), stop=(ci == NCH - 1))
                    kt = small.tile([128, 512], BF16, tag="ktmp", bufs=2)
                    nc.scalar.activation(kt[:], p[:], AF.Identity,
                                         bias=bias_sb["bk"][:, mo:mo + 1])
                    nc.gpsimd.dma_start(kts_d[b, mo, :, n * 512:(n + 1) * 512],
                                        kt[:])

            # ---- v projection (token-major, bf16); kin slot reused ----
            wv_q = [load_wq("wv", qo) for qo in range(4)]
            vkm = big.tile([128, NCH, E], BF16, tag="kin_vkm", name=f"vkm{b}")
            for kc in range(NCH):
                keyc = small.tile([128, NCH, 128], F32R, tag="keyc", bufs=1)
                nc.gpsimd.dma_start(
                    keyc[:],
                    keyT_d[b][:, kc * 128:(kc + 1) * 128]
                    .rearrange("(c p) t -> p c t", p=128))
                for q4 in range(4):
                    p = ps_half.tile([128, 256], F32, tag="half")
                    for ci in range(NCH):
                        nc.tensor.matmul(
                            p[:], keyc[:, ci, :], wv_q[q4][:, ci, :],
                            start=(ci == 0), stop=(ci == NCH - 1))
                    nc.vector.tensor_add(
                        vkm[:, kc, q4 * 256:(q4 + 1) * 256], p[:],
                        bvb[:, q4 * 256:(q4 + 1) * 256])

            # ---- attention: software-pipelined over 2-head groups ----
            awA = med.tile([128, NCH, QL], F32, tag="awA", name=f"awA{b}")
            aoutT[b] = med.tile([128, NCH, QL], F32R, tag="aoutT",
                                bufs=2, name=f"aoutT{b}")
            pTs = [None] * NCH

            def front(g):
                ktd = med.tile([128, L], BF16, tag="ktd", bufs=2,
                               name=f"ktd{b}_{g}")
                nc.scalar.dma_start(ktd[:], kts_d[b, g])
                s = g % 2
                nc.vector.tensor_copy(qzd[0:64, s, 0, :],
                                      qT[0:64, g, :].bitcast(F32))
                nc.vector.tensor_copy(qzd[64:128, s, 1, :],
                                      qT[64:128, g, :].bitcast(F32))
                pT = med.tile([128, NCH, 2, QL], BF16, tag="pT", bufs=2,
                              name=f"pT{b}_{g}")
                pTs[g] = pT
                for kc in range(NCH):
                    ps = ps_proj.tile([128, 2, QL], F32, tag="proj")
                    nc.tensor.matmul(
                        ps[:].rearrange("p a b -> p (a b)"),
                        ktd[:, kc * 128:(kc + 1) * 128],
                        qzd[:, s].rearrange("p a b -> p (a b)"),
                        start=True, stop=True)
                    nc.scalar.activation(pT[:, kc], ps[:], AF.Exp, scale=0.125)
                # coarse mask over all kc / both halves (bf16, 2x DVE)
                nc.vector.tensor_mul(pT[:], pT[:],
                                     _bcast_mid(maskt[:], 2, 2))

            def back(g):
                pT = pTs[g]
                # rowsum of masked exp, broadcast via ones-matmul
                rp = ps_proj.tile([128, 2, QL], F32, tag="proj")
                for kc in range(NCH):
                    nc.tensor.matmul(rp[:].rearrange("p a b -> p (a b)"),
                                     ones_t[:],
                                     pT[:, kc].rearrange("p a b -> p (a b)"),
                                     start=(kc == 0), stop=(kc == NCH - 1))
                r1f = small.tile([1, 2, QL], F32, tag="r1f", bufs=1)
                nc.vector.reciprocal_approx_fast(r1f[:], rp[0:1, :, :])
                r1 = small.tile([1, 2, QL], BF16, tag="r1", bufs=2)
                nc.vector.tensor_copy(r1[:], r1f[:])

                # attn @ V per head (M=64) on unnormalized masked pT
                pav = [ps_half.tile([64, QL], F32, tag="half",
                                    name=f"pav{hi}") for hi in range(2)]
                for kc in range(NCH):
                    for hi in range(2):
                        h = 2 * g + hi
                        nc.tensor.matmul(
                            pav[hi][:, :],
                            vkm[:, kc, h * 64:(h + 1) * 64],
                            pT[:, kc, hi, :],
                            start=(kc == 0), stop=(kc == NCH - 1))

                # broadcast reciprocal to all partitions
                rbp = ps_proj.tile([128, 2, QL], F32, tag="proj")
                nc.tensor.matmul(rbp[:].rearrange("p a b -> p (a b)"),
                                 ones_t[0:1, :],
                                 r1[:].rearrange("p a b -> p (a b)"),
                                 start=True, stop=True)
                rb16 = small.tile([128, 2, QL], BF16, tag="rb16", bufs=2)
                nc.scalar.activation(rb16[:], rbp[:], AF.Copy)
                recipb = small.tile([64, 2, QL], F32, tag="recipb", bufs=2)
                nc.vector.tensor_copy(recipb[:], rbp[0:64])

                # normalize attn@V during eviction; odd head partition-shifted
                nc.vector.tensor_mul(aoutT[b][0:64, g, :], pav[0][:, :],
                                     recipb[:, 0, :])
                sh = small.tile([64, QL], F32R, tag="btmp")
                nc.vector.tensor_mul(sh[:, :], pav[1][:, :],
                                     recipb[:, 1, :])
                nc.sync.dma_start(aoutT[b][64:128, g, :], sh[:, :])

                # attn_w: normalize pT in place (all-bf16 2x; attn@V already
                # consumed the unnormalized values), then f32 accumulate
                nc.vector.tensor_mul(pT[:], pT[:],
                                     _bcast_mid(rb16[:], NCH, 1))
                if g == 0:
                    nc.vector.tensor_copy(awA[:], pT[:, :, 0, :])
                else:
                    nc.vector.tensor_add(awA[:], awA[:], pT[:, :, 0, :])
                nc.vector.tensor_add(awA[:], awA[:], pT[:, :, 1, :])

            for g in range(NCH):
                front(g)
                if g > 0:
                    back(g - 1)
            back(NCH - 1)

            nc.gpsimd.tensor_scalar_mul(awA[:], awA[:], 1.0 / H)
            nc.sync.dma_start(
                attnwT_d[b].rearrange("(c p) t -> p c t", p=128), awA[:])

            # ---- out projection + relu + residual (f32r) ----
            wo_q = [load_wq("wo", qo) for qo in range(4)]
            oriq_a = load_oriq(b, "a")
            outT[b] = med.tile([128, NCH, QL], F32R, tag="outT",
                               bufs=B_PER_CORE, name=f"outT{b}")
            for mo in range(NCH):
                p = ps_half.tile([128, QL], F32, tag="half")
                wt = wo_q[mo // 2]
                for ci in range(NCH):
                    nc.tensor.matmul(
                        p[:], wt[:, ci, (mo % 2) * 128:(mo % 2) * 128 + 128],
                        aoutT[b][:, ci, :], start=(ci == 0), stop=(ci == NCH - 1))
                t = small.tile([128, QL], F32, tag="btmp")
                nc.scalar.activation(t[:], p[:], AF.Relu,
                                     bias=bias_sb["bo"][:, mo:mo + 1])
                nc.vector.tensor_add(outT[b][:, mo, :], t[:],
                                     oriq_a[:, mo, :].bitcast(F32))

        # ================= PHASE B: gated GRU (both batches) =================
        rqT = [None] * B_PER_CORE
        for b in range(B_PER_CORE):
            rqT[b] = med.tile([128, NCH, QL], F32R, tag="qrh", bufs=2, name=f"rqT{b}")
        hT = [None] * B_PER_CORE

        for stage, (wname, bias, func) in enumerate(
                [("wrC", "brz", AF.Relu), ("wgC", "bgg", AF.Tanh),
                 ("wzC", "bzz", AF.Relu)]):
            oriq_g = ([load_oriq(b, f"g{stage}") for b in range(B_PER_CORE)]
                      if stage != 1 else [None] * B_PER_CORE)
            if stage == 1:
                for b in range(B_PER_CORE):
                    hT[b] = med.tile([128, NCH, QL], F32, tag="aoutT",
                                     bufs=2, name=f"hT{b}")
            for mo in range(NCH):
                w8 = load_w8(wname, mo)
                for b in range(B_PER_CORE):
                    p = ps_half.tile([128, QL], F32, tag="half")
                    xside = oriq_g[b] if stage != 1 else rqT[b]
                    for ci in range(2 * NCH):
                        rhs = (xside[:, ci, :] if ci < NCH
                               else outT[b][:, ci - NCH, :])
                        nc.tensor.matmul(
                            p[:], w8[:, ci, :], rhs,
                            start=(ci == 0), stop=(ci == 2 * NCH - 1))
                    if stage == 0:   # r -> rq
                        t = small.tile([128, QL], F32, tag="btmp")
                        nc.scalar.activation(t[:], p[:], func,
                                             bias=bias_sb[bias][:, mo:mo + 1])
                        nc.vector.tensor_mul(rqT[b][:, mo, :], t[:],
                                             oriq_g[b][:, mo, :].bitcast(F32))
                    elif stage == 1:  # h
                        nc.scalar.activation(hT[b][:, mo, :], p[:], func,
                                             bias=bias_sb[bias][:, mo:mo + 1])
                    else:            # z + final blend + store
                        zt = small.tile([128, QL], F32, tag="btmp")
                        nc.scalar.activation(zt[:], p[:], func,
                                             bias=bias_sb[bias][:, mo:mo + 1])
                        d = small.tile([128, QL], F32, tag="btmp")
                        nc.vector.tensor_sub(d[:], hT[b][:, mo, :],
                                             oriq_g[b][:, mo, :].bitcast(F32))
                        nc.vector.tensor_mul(d[:], d[:], zt[:])
                        fin = small.tile([128, QL], F32, tag="btmp")
                        nc.vector.tensor_add(fin[:], d[:],
                                             oriq_g[b][:, mo, :].bitcast(F32))
                        nc.sync.dma_start(
                            outT_d[b][mo * 128:(mo + 1) * 128, :], fin[:])


def prep_inputs_core(core, key, pe, key_index, key_padding_mask,
                     in_proj_w, in_proj_b, out_w, out_b, gw, gb):
    b0 = core * B_PER_CORE
    sl = slice(b0, b0 + B_PER_CORE)
    import ml_dtypes as _mld
    keyc = np.asarray(key[sl], np.float32)
    kin = keyc + np.asarray(pe[sl], np.float32)
    kinT = np.ascontiguousarray(kin.transpose(0, 2, 1)).astype(_mld.bfloat16)
    keyT = np.ascontiguousarray(keyc.transpose(0, 2, 1))

    ki = np.asarray(key_index[sl])
    pad = np.asarray(key_padding_mask[sl])
    qi = ki[:, L - QL:]
    ri = ki[:, :L - QL]
    import ml_dtypes
    allowed = np.zeros((B_PER_CORE, L, QL), np.float32)
    allowed[:, :L - QL, :] = ((ri[:, :, None] < qi[:, None, :])
                              & ~pad[:, :L - QL, None])
    allowed[:, L - QL:, :] = np.eye(QL, dtype=np.float32)[None]

    w32 = lambda x: np.asarray(x, np.float32)
    im = {
        "kinT": kinT, "keyT": keyT,
        "maskT": allowed.astype(ml_dtypes.bfloat16),
        "ones": np.ones((128, 128), ml_dtypes.bfloat16),
        "bv": w32(in_proj_b[2 * E:]).reshape(1, E),
        "bq": w32(in_proj_b[:E]),
        "bk": w32(in_proj_b[E:2 * E]),
        "bo": w32(out_b),
        "brz": w32(gb["bxr"] + gb["byr"]),
        "bzz": w32(gb["bxz"] + gb["byz"]),
        "bgg": w32(gb["bxg"] + gb["byg"]),
        "wqT": np.ascontiguousarray(w32(in_proj_w[:E]).T).astype(_mld.bfloat16),
        "wkT": np.ascontiguousarray(w32(in_proj_w[E:2 * E]).T).astype(_mld.bfloat16),
        "wvT": np.ascontiguousarray(w32(in_proj_w[2 * E:]).T),
        "woT": np.ascontiguousarray(w32(out_w).T),
        "wrC": np.ascontiguousarray(
            np.concatenate([w32(gw["wxr"]).T, w32(gw["wyr"]).T], 0)),
        "wzC": np.ascontiguousarray(
            np.concatenate([w32(gw["wxz"]).T, w32(gw["wyz"]).T], 0)),
        "wgC": np.ascontiguousarray(
            np.concatenate([w32(gw["wxg"]).T, w32(gw["wyg"]).T], 0)),
    }
    return im


def postprocess(results):
    outs, aws = [], []
    for r in results:
        outs.append(r["outT"].transpose(0, 2, 1))
        aws.append(r["attnwT"].transpose(0, 2, 1))
    return (np.concatenate(outs, 0), np.concatenate(aws, 0))


_NC_CACHE = {}


def kernel(key, pe, key_index, key_padding_mask, query_length,
           in_proj_w, in_proj_b, out_w, out_b,
           wxr, bxr, wyr, byr, wxz, bxz, wyz, byz, wxg, bxg, wyg, byg):
    """Full-input entry point: shard B=16 across 8 NeuronCores, run, gather."""
    from concourse.bass_utils import run_bass_kernel_spmd

    key = np.asarray(key)
    assert int(query_length) == QL and key.shape == (16, L, E)
    if "nc" not in _NC_CACHE:
        _NC_CACHE["nc"] = build_kernel(num_devices=8)
    nc = _NC_CACHE["nc"]

    gw = {"wxr": wxr, "wyr": wyr, "wxz": wxz, "wyz": wyz,
          "wxg": wxg, "wyg": wyg}
    gb = {"bxr": bxr, "byr": byr, "bxz": bxz, "byz": byz,
          "bxg": bxg, "byg": byg}
    in_maps = [prep_inputs_core(c, key, pe, key_index, key_padding_mask,
                                in_proj_w, in_proj_b, out_w, out_b, gw, gb)
               for c in range(8)]
    res = run_bass_kernel_spmd(nc, in_maps, core_ids=list(range(8)))
    out, attn_w = postprocess(res.results)
    return out.astype(np.float32), attn_w.astype(np.float32)


# revision 28
# speedup vs baseline: 1.2008x; 1.1186x over previous
"""Bass/Tile kernel for EpisodeMultiheadAttentionBlock on TRN2.

Per-core: 2 batch elements (data-parallel over B=16 across 8 cores).

Layout: activations feature-major [feature parts, token free]; V token-major.
Projections/GRU matmuls float32r (1 cyc/row at free>=256); attention island
(scores, exp/pT, V, rowsum) in bf16 for 2x DVE + half DMA. Softmax without
max-subtraction; multiplicative 0/1 mask applied as ONE coarse op per head
group; 1/rowsum via ones-matmul broadcast; attn@V consumes unnormalized pT
(normalized on the [64,QL] eviction); attn_w accumulated in f32 from a
Pool-engine coarse product. Attention groups are software-pipelined
(scores/exp of group g+1 issued before the reduction half of group g).
"""

import numpy as np

import concourse.bass as bass
import concourse.mybir as mybir
import concourse.tile as tile
from concourse import bacc

F32 = mybir.dt.float32
F32R = mybir.dt.float32r
BF16 = mybir.dt.bfloat16
AF = mybir.ActivationFunctionType

B_PER_CORE = 2
L = 1024
E = 1024
H = 16
QL = 256
NCH = 8

SINGLES = ["wq", "wk", "wv", "wo"]
GRUCATS = ["wrC", "wzC", "wgC"]


def build_kernel(num_devices=8):
    nc = bacc.Bacc("TRN2", target_bir_lowering=False, debug=False,
                   num_devices=num_devices)

    kinT_d = nc.dram_tensor("kinT", [B_PER_CORE, E, L], BF16, kind="ExternalInput")
    keyT_d = nc.dram_tensor("keyT", [B_PER_CORE, E, L], F32R, kind="ExternalInput")
    maskT_d = nc.dram_tensor("maskT", [B_PER_CORE, L, QL], BF16, kind="ExternalInput")
    ones_d = nc.dram_tensor("ones", [128, 128], BF16, kind="ExternalInput")
    w_d = {n: nc.dram_tensor(n + "T", [E, E],
                             BF16 if n in ("wq", "wk") else F32R,
                             kind="ExternalInput")
           for n in SINGLES}
    for n in GRUCATS:
        w_d[n] = nc.dram_tensor(n, [2 * E, E], F32R, kind="ExternalInput")
    b_d = {n: nc.dram_tensor(n, [E], F32, kind="ExternalInput")
           for n in ["bq", "bk", "bo", "brz", "bzz", "bgg"]}
    bv_d = nc.dram_tensor("bv", [1, E], F32, kind="ExternalInput")

    kts_d = nc.dram_tensor("kts", [B_PER_CORE, NCH, 128, L], BF16)
    outT_d = nc.dram_tensor("outT", [B_PER_CORE, E, QL], F32, kind="ExternalOutput")
    attnwT_d = nc.dram_tensor("attnwT", [B_PER_CORE, L, QL], F32,
                              kind="ExternalOutput")

    with tile.TileContext(nc) as tc, nc.allow_low_precision(
            reason="bf16 attention island feeds f32 PSUM; f32 accumulators"):
        _body(nc, tc, kinT_d, keyT_d, maskT_d, ones_d, w_d, b_d, bv_d,
              kts_d, outT_d, attnwT_d)
    nc.compile()
    return nc


def _bcast_mid(ap, n, pos):
    """Insert a step-0 dim of extent n at position pos of an AP."""
    lst = [list(p) for p in ap.ap]
    return bass.AP(tensor=ap.tensor, offset=ap.offset,
                   ap=lst[:pos] + [[0, n]] + lst[pos:])


def _body(nc, tc, kinT_d, keyT_d, maskT_d, ones_d, w_d, b_d, bv_d,
          kts_d, outT_d, attnwT_d):
    from contextlib import ExitStack
    ctx = ExitStack()
    with ctx:
        consts = ctx.enter_context(tc.tile_pool(name="consts", bufs=1))
        wpool = ctx.enter_context(tc.tile_pool(name="wpool", bufs=4))
        big = ctx.enter_context(tc.tile_pool(name="big", bufs=1))
        med = ctx.enter_context(tc.tile_pool(name="med", bufs=1))
        small = ctx.enter_context(tc.tile_pool(name="small", bufs=2))
        ps_proj = ctx.enter_context(
            tc.tile_pool(name="ps_proj", bufs=5, space="PSUM"))
        ps_half = ctx.enter_context(
            tc.tile_pool(name="ps_half", bufs=3, space="PSUM"))

        ones_t = consts.tile([128, 128], BF16, tag="ones")
        nc.sync.dma_start(ones_t[:], ones_d[:])
        bias_sb = {}
        for n, d in b_d.items():
            t = consts.tile([128, NCH], F32, tag="b_" + n)
            nc.sync.dma_start(t[:], d.rearrange("(c p) -> p c", p=128))
            bias_sb[n] = t
        bvb = consts.tile([128, E], F32, tag="bvb")
        bv_ap = bv_d[:, :]
        nc.gpsimd.dma_start(
            out=bvb[:],
            in_=bass.AP(tensor=bv_ap.tensor, offset=bv_ap.offset,
                        ap=[[0, 128], [1, E]]))

        # persistent double-buffered query staging; zero halves set once
        qzd = consts.tile([128, 2, 2, QL], BF16, tag="qzd")
        nc.gpsimd.memset(qzd[64:128, 0, 0, :], 0.0)
        nc.gpsimd.memset(qzd[0:64, 0, 1, :], 0.0)
        nc.gpsimd.memset(qzd[64:128, 1, 0, :], 0.0)
        nc.gpsimd.memset(qzd[0:64, 1, 1, :], 0.0)

        def load_wq(name, qo, dt=F32R):
            """Quarter qo (out-cols qo*256..+256) of a single [E,E] weight."""
            t = wpool.tile([128, NCH, 256], dt, tag="w", name=f"{name}_{qo}")
            nc.sync.dma_start(
                t[:],
                w_d[name][:, qo * 256:(qo + 1) * 256]
                .rearrange("(c p) o -> p c o", p=128))
            return t

        def load_w8(name, mo):
            """Eighth mo (out-cols mo*128..+128) of a [2E,E] GRU cat weight."""
            t = wpool.tile([128, 2 * NCH, 128], F32R, tag="w",
                           name=f"{name}_{mo}")
            nc.sync.dma_start(
                t[:],
                w_d[name][:, mo * 128:(mo + 1) * 128]
                .rearrange("(c p) o -> p c o", p=128))
            return t

        def load_oriq(b, phase):
            t = med.tile([128, NCH, QL], F32R, tag="oriq", bufs=2,
                         name=f"oriq_{phase}_{b}")
            nc.sync.dma_start(
                t[:], keyT_d[b][:, L - QL:].rearrange("(c p) t -> p c t", p=128))
            return t

        outT = [None] * B_PER_CORE
        aoutT = [None] * B_PER_CORE

        # ================= PHASE A (per batch) =================
        for b in range(B_PER_CORE):
            # weights first on the sync queue so the first matmul starts early
            wq_q = [load_wq("wq", qo, BF16) for qo in range(4)]
            kin = big.tile([128, NCH, L], BF16, tag="kin_vkm", name=f"kin{b}")
            kin_re = kinT_d[b].rearrange("(c p) t -> p c t", p=128)
            # q-projection token slice first so PE can start ASAP
            nc.sync.dma_start(kin[:, :, L - QL:], kin_re[:, :, L - QL:])
            nc.sync.dma_start(kin[:, :, 0:512], kin_re[:, :, 0:512])
            nc.sync.dma_start(kin[:, :, 512:L - QL], kin_re[:, :, 512:L - QL])
            maskt = big.tile([128, NCH, QL], BF16, tag="mask", name=f"mask{b}")
            nc.scalar.dma_start(maskt[:], maskT_d[b].rearrange("(c p) t -> p c t", p=128))
            qT = med.tile([128, NCH, QL], F32R, tag="qrh", bufs=2, name=f"qT{b}")
            for mo in range(NCH):
                p = ps_half.tile([128, QL], F32, tag="half")
                wt = wq_q[mo // 2]
                for ci in range(NCH):
                    nc.tensor.matmul(
                        p[:], wt[:, ci, (mo % 2) * 128:(mo % 2) * 128 + 128],
                        kin[:, ci, L - QL:],
                        start=(ci == 0), stop=(ci == NCH - 1))
                nc.scalar.activation(qT[:, mo, :], p[:], AF.Identity,
                                     bias=bias_sb["bq"][:, mo:mo + 1])

            # ---- k projection (streamed to DRAM scratch as bf16) ----
            wk_q = [load_wq("wk", qo, BF16) for qo in range(4)]
            for mo in range(NCH):
                wt = wk_q[mo // 2]
                for n in range(2):
                    p = ps_proj.tile([128, 512], F32, tag="proj")
                    for ci in range(NCH):
                        nc.tensor.matmul(
                            p[:], wt[:, ci, (mo % 2) * 128:(mo % 2) * 128 + 128],
                            kin[:, ci, n * 512:(n + 1) * 512],
                            start=(ci == 0), stop=(ci == NCH - 1))
                    kt = small.tile([128, 512], BF16, tag="ktmp", bufs=2)
                    nc.scalar.activation(kt[:], p[:], AF.Identity,
                                         bias=bias_sb["bk"][:, mo:mo + 1])
                    nc.gpsimd.dma_start(kts_d[b, mo, :, n * 512:(n + 1) * 512],
                                        kt[:])

            # ---- v projection (token-major, bf16); kin slot reused ----
            wv_q = [load_wq("wv", qo) for qo in range(4)]
            vkm = big.tile([128, NCH, E], BF16, tag="kin_vkm", name=f"vkm{b}")
            for kc in range(NCH):
                keyc = small.tile([128, NCH, 128], F32R, tag="keyc", bufs=1)
                nc.gpsimd.dma_start(
                    keyc[:],
                    keyT_d[b][:, kc * 128:(kc + 1) * 128]
                    .rearrange("(c p) t -> p c t", p=128))
                for q4 in range(4):
                    p = ps_half.tile([128, 256], F32, tag="half")
                    for ci in range(NCH):
                        nc.tensor.matmul(
                            p[:], keyc[:, ci, :], wv_q[q4][:, ci, :],
                            start=(ci == 0), stop=(ci == NCH - 1))
                    nc.vector.tensor_add(
                        vkm[:, kc, q4 * 256:(q4 + 1) * 256], p[:],
                        bvb[:, q4 * 256:(q4 + 1) * 256])

            # ---- attention: software-pipelined over 2-head groups ----
            awA = med.tile([128, NCH, QL], F32, tag="awA", name=f"awA{b}")
            aoutT[b] = med.tile([128, NCH, QL], F32R, tag="aoutT",
                                bufs=2, name=f"aoutT{b}")
            pTs = [None] * NCH

            def front(g):
                ktd = med.tile([128, L], BF16, tag="ktd", bufs=2,
                               name=f"ktd{b}_{g}")
                nc.scalar.dma_start(ktd[:], kts_d[b, g])
                s = g % 2
                nc.vector.tensor_copy(qzd[0:64, s, 0, :],
                                      qT[0:64, g, :].bitcast(F32))
                nc.vector.tensor_copy(qzd[64:128, s, 1, :],
                                      qT[64:128, g, :].bitcast(F32))
                pT = med.tile([128, NCH, 2, QL], BF16, tag="pT", bufs=3,
                              name=f"pT{b}_{g}")
                pTs[g] = pT
                for kc in range(NCH):
                    ps = ps_proj.tile([128, 2, QL], F32, tag="proj")
                    nc.tensor.matmul(
                        ps[:].rearrange("p a b -> p (a b)"),
                        ktd[:, kc * 128:(kc + 1) * 128],
                        qzd[:, s].rearrange("p a b -> p (a b)"),
                        start=True, stop=True)
                    nc.scalar.activation(pT[:, kc], ps[:], AF.Exp, scale=0.125)
                # coarse mask over all kc / both halves (bf16, 2x DVE)
                nc.vector.tensor_mul(pT[:], pT[:],
                                     _bcast_mid(maskt[:], 2, 2))

            def back(g):
                pT = pTs[g]
                # rowsum of masked exp, broadcast via ones-matmul
                rp = ps_proj.tile([128, 2, QL], F32, tag="proj")
                for kc in range(NCH):
                    nc.tensor.matmul(rp[:].rearrange("p a b -> p (a b)"),
                                     ones_t[:],
                                     pT[:, kc].rearrange("p a b -> p (a b)"),
                                     start=(kc == 0), stop=(kc == NCH - 1))
                r1f = small.tile([1, 2, QL], F32, tag="r1f", bufs=1)
                nc.vector.reciprocal_approx_fast(r1f[:], rp[0:1, :, :])
                r1 = small.tile([1, 2, QL], BF16, tag="r1", bufs=2)
                nc.vector.tensor_copy(r1[:], r1f[:])

                # attn @ V per head (M=64) on unnormalized masked pT
                pav = [ps_half.tile([64, QL], F32, tag="half",
                                    name=f"pav{hi}") for hi in range(2)]
                for kc in range(NCH):
                    for hi in range(2):
                        h = 2 * g + hi
                        nc.tensor.matmul(
                            pav[hi][:, :],
                            vkm[:, kc, h * 64:(h + 1) * 64],
                            pT[:, kc, hi, :],
                            start=(kc == 0), stop=(kc == NCH - 1))

                # broadcast reciprocal to all partitions
                rbp = ps_proj.tile([128, 2, QL], F32, tag="proj")
                nc.tensor.matmul(rbp[:].rearrange("p a b -> p (a b)"),
                                 ones_t[0:1, :],
                                 r1[:].rearrange("p a b -> p (a b)"),
                                 start=True, stop=True)
                rb16 = small.tile([128, 2, QL], BF16, tag="rb16", bufs=2)
                nc.scalar.activation(rb16[:], rbp[:], AF.Copy)
                recipb = small.tile([64, 2, QL], F32, tag="recipb", bufs=2)
                nc.vector.tensor_copy(recipb[:], rbp[0:64])

                # normalize attn@V during eviction; odd head partition-shifted
                nc.vector.tensor_mul(aoutT[b][0:64, g, :], pav[0][:, :],
                                     recipb[:, 0, :])
                sh = small.tile([64, QL], F32R, tag="btmp")
                nc.vector.tensor_mul(sh[:, :], pav[1][:, :],
                                     recipb[:, 1, :])
                nc.sync.dma_start(aoutT[b][64:128, g, :], sh[:, :])

                # attn_w: normalize pT in place (all-bf16 2x; attn@V already
                # consumed the unnormalized values), then f32 accumulate
                nc.vector.tensor_mul(pT[:], pT[:],
                                     _bcast_mid(rb16[:], NCH, 1))
                if g == 0:
                    nc.vector.tensor_copy(awA[:], pT[:, :, 0, :])
                else:
                    nc.vector.tensor_add(awA[:], awA[:], pT[:, :, 0, :])
                nc.vector.tensor_add(awA[:], awA[:], pT[:, :, 1, :])

            for g in range(NCH):
                front(g)
                if g > 0:
                    back(g - 1)
            back(NCH - 1)

            nc.gpsimd.tensor_scalar_mul(awA[:], awA[:], 1.0 / H)
            nc.sync.dma_start(
                attnwT_d[b].rearrange("(c p) t -> p c t", p=128), awA[:])

            # ---- out projection + relu + residual (f32r) ----
            wo_q = [load_wq("wo", qo) for qo in range(4)]
            oriq_a = load_oriq(b, "a")
            outT[b] = med.tile([128, NCH, QL], F32R, tag="outT",
                               bufs=B_PER_CORE, name=f"outT{b}")
            for mo in range(NCH):
                p = ps_half.tile([128, QL], F32, tag="half")
                wt = wo_q[mo // 2]
                for ci in range(NCH):
                    nc.tensor.matmul(
                        p[:], wt[:, ci, (mo % 2) * 128:(mo % 2) * 128 + 128],
                        aoutT[b][:, ci, :], start=(ci == 0), stop=(ci == NCH - 1))
                t = small.tile([128, QL], F32, tag="btmp")
                nc.scalar.activation(t[:], p[:], AF.Relu,
                                     bias=bias_sb["bo"][:, mo:mo + 1])
                nc.vector.tensor_add(outT[b][:, mo, :], t[:],
                                     oriq_a[:, mo, :].bitcast(F32))

        # ================= PHASE B: gated GRU (both batches) =================
        rqT = [None] * B_PER_CORE
        for b in range(B_PER_CORE):
            rqT[b] = med.tile([128, NCH, QL], F32R, tag="qrh", bufs=2, name=f"rqT{b}")
        hT = [None] * B_PER_CORE

        for stage, (wname, bias, func) in enumerate(
                [("wrC", "brz", AF.Relu), ("wgC", "bgg", AF.Tanh),
                 ("wzC", "bzz", AF.Relu)]):
            oriq_g = ([load_oriq(b, f"g{stage}") for b in range(B_PER_CORE)]
                      if stage != 1 else [None] * B_PER_CORE)
            if stage == 1:
                for b in range(B_PER_CORE):
                    hT[b] = med.tile([128, NCH, QL], F32, tag="aoutT",
                                     bufs=2, name=f"hT{b}")
            for mo in range(NCH):
                w8 = load_w8(wname, mo)
                for b in range(B_PER_CORE):
                    p = ps_half.tile([128, QL], F32, tag="half")
                    xside = oriq_g[b] if stage != 1 else rqT[b]
                    for ci in range(2 * NCH):
                        rhs = (xside[:, ci, :] if ci < NCH
                               else outT[b][:, ci - NCH, :])
                        nc.tensor.matmul(
                            p[:], w8[:, ci, :], rhs,
                            start=(ci == 0), stop=(ci == 2 * NCH - 1))
                    if stage == 0:   # r -> rq
                        t = small.tile([128, QL], F32, tag="btmp")
                        nc.scalar.activation(t[:], p[:], func,
                                             bias=bias_sb[bias][:, mo:mo + 1])
                        nc.vector.tensor_mul(rqT[b][:, mo, :], t[:],
                                             oriq_g[b][:, mo, :].bitcast(F32))
                    elif stage == 1:  # h
                        nc.scalar.activation(hT[b][:, mo, :], p[:], func,
                                             bias=bias_sb[bias][:, mo:mo + 1])
                    else:            # z + final blend + store
                        zt = small.tile([128, QL], F32, tag="btmp")
                        nc.scalar.activation(zt[:], p[:], func,
                                             bias=bias_sb[bias][:, mo:mo + 1])
                        d = small.tile([128, QL], F32, tag="btmp")
                        nc.vector.tensor_sub(d[:], hT[b][:, mo, :],
                                             oriq_g[b][:, mo, :].bitcast(F32))
                        nc.vector.tensor_mul(d[:], d[:], zt[:])
                        fin = small.tile([128, QL], F32, tag="btmp")
                        nc.vector.tensor_add(fin[:], d[:],
                                             oriq_g[b][:, mo, :].bitcast(F32))
                        nc.sync.dma_start(
                            outT_d[b][mo * 128:(mo + 1) * 128, :], fin[:])


def prep_inputs_core(core, key, pe, key_index, key_padding_mask,
                     in_proj_w, in_proj_b, out_w, out_b, gw, gb):
    b0 = core * B_PER_CORE
    sl = slice(b0, b0 + B_PER_CORE)
    import ml_dtypes as _mld
    keyc = np.asarray(key[sl], np.float32)
    kin = keyc + np.asarray(pe[sl], np.float32)
    kinT = np.ascontiguousarray(kin.transpose(0, 2, 1)).astype(_mld.bfloat16)
    keyT = np.ascontiguousarray(keyc.transpose(0, 2, 1))

    ki = np.asarray(key_index[sl])
    pad = np.asarray(key_padding_mask[sl])
    qi = ki[:, L - QL:]
    ri = ki[:, :L - QL]
    import ml_dtypes
    allowed = np.zeros((B_PER_CORE, L, QL), np.float32)
    allowed[:, :L - QL, :] = ((ri[:, :, None] < qi[:, None, :])
                              & ~pad[:, :L - QL, None])
    allowed[:, L - QL:, :] = np.eye(QL, dtype=np.float32)[None]

    w32 = lambda x: np.asarray(x, np.float32)
    im = {
        "kinT": kinT, "keyT": keyT,
        "maskT": allowed.astype(ml_dtypes.bfloat16),
        "ones": np.ones((128, 128), ml_dtypes.bfloat16),
        "bv": w32(in_proj_b[2 * E:]).reshape(1, E),
        "bq": w32(in_proj_b[:E]),
        "bk": w32(in_proj_b[E:2 * E]),
        "bo": w32(out_b),
        "brz": w32(gb["bxr"] + gb["byr"]),
        "bzz": w32(gb["bxz"] + gb["byz"]),
        "bgg": w32(gb["bxg"] + gb["byg"]),
        "wqT": np.ascontiguousarray(w32(in_proj_w[:E]).T).astype(_mld.bfloat16),
        "wkT": np.ascontiguousarray(w32(in_proj_w[E:2 * E]).T).astype(_mld.bfloat16),
        "wvT": np.ascontiguousarray(w32(in_proj_w[2 * E:]).T),
        "woT": np.ascontiguousarray(w32(out_w).T),
        "wrC": np.ascontiguousarray(
            np.concatenate([w32(gw["wxr"]).T, w32(gw["wyr"]).T], 0)),
        "wzC": np.ascontiguousarray(
            np.concatenate([w32(gw["wxz"]).T, w32(gw["wyz"]).T], 0)),
        "wgC": np.ascontiguousarray(
            np.concatenate([w32(gw["wxg"]).T, w32(gw["wyg"]).T], 0)),
    }
    return im


def postprocess(results):
    outs, aws = [], []
    for r in results:
        outs.append(r["outT"].transpose(0, 2, 1))
        aws.append(r["attnwT"].transpose(0, 2, 1))
    return (np.concatenate(outs, 0), np.concatenate(aws, 0))


_NC_CACHE = {}


def kernel(key, pe, key_index, key_padding_mask, query_length,
           in_proj_w, in_proj_b, out_w, out_b,
           wxr, bxr, wyr, byr, wxz, bxz, wyz, byz, wxg, bxg, wyg, byg):
    """Full-input entry point: shard B=16 across 8 NeuronCores, run, gather."""
    from concourse.bass_utils import run_bass_kernel_spmd

    key = np.asarray(key)
    assert int(query_length) == QL and key.shape == (16, L, E)
    if "nc" not in _NC_CACHE:
        _NC_CACHE["nc"] = build_kernel(num_devices=8)
    nc = _NC_CACHE["nc"]

    gw = {"wxr": wxr, "wyr": wyr, "wxz": wxz, "wyz": wyz,
          "wxg": wxg, "wyg": wyg}
    gb = {"bxr": bxr, "byr": byr, "bxz": bxz, "byz": byz,
          "bxg": bxg, "byg": byg}
    in_maps = [prep_inputs_core(c, key, pe, key_index, key_padding_mask,
                                in_proj_w, in_proj_b, out_w, out_b, gw, gb)
               for c in range(8)]
    res = run_bass_kernel_spmd(nc, in_maps, core_ids=list(range(8)))
    out, attn_w = postprocess(res.results)
    return out.astype(np.float32), attn_w.astype(np.float32)


# revision 31
# speedup vs baseline: 1.2010x; 1.0002x over previous
"""Bass/Tile kernel for EpisodeMultiheadAttentionBlock on TRN2.

Per-core: 2 batch elements (data-parallel over B=16 across 8 cores).

Layout: activations feature-major [feature parts, token free]; V token-major.
Projections/GRU matmuls float32r (1 cyc/row at free>=256); attention island
(scores, exp/pT, V, rowsum) in bf16 for 2x DVE + half DMA. Softmax without
max-subtraction; multiplicative 0/1 mask applied as ONE coarse op per head
group; 1/rowsum via ones-matmul broadcast; attn@V consumes unnormalized pT
(normalized on the [64,QL] eviction); attn_w accumulated in f32 from a
Pool-engine coarse product. Attention groups are software-pipelined
(scores/exp of group g+1 issued before the reduction half of group g).
"""

import numpy as np

import concourse.bass as bass
import concourse.mybir as mybir
import concourse.tile as tile
from concourse import bacc

F32 = mybir.dt.float32
F32R = mybir.dt.float32r
BF16 = mybir.dt.bfloat16
AF = mybir.ActivationFunctionType

B_PER_CORE = 2
L = 1024
E = 1024
H = 16
QL = 256
NCH = 8

SINGLES = ["wq", "wk", "wv", "wo"]
GRUCATS = ["wrC", "wzC", "wgC"]


def build_kernel(num_devices=8):
    nc = bacc.Bacc("TRN2", target_bir_lowering=False, debug=False,
                   num_devices=num_devices)

    kinT_d = nc.dram_tensor("kinT", [B_PER_CORE, E, L], BF16, kind="ExternalInput")
    keyT_d = nc.dram_tensor("keyT", [B_PER_CORE, E, L], F32R, kind="ExternalInput")
    maskT_d = nc.dram_tensor("maskT", [B_PER_CORE, L, QL], BF16, kind="ExternalInput")
    ones_d = nc.dram_tensor("ones", [128, 128], BF16, kind="ExternalInput")
    # weights pre-swizzled on host: [p, chunk, c, o] so DMA lines are dense
    w_d = {n: nc.dram_tensor(n + "T", [128, 4, NCH, 256],
                             BF16 if n in ("wq", "wk") else F32R,
                             kind="ExternalInput")
           for n in SINGLES}
    for n in GRUCATS:
        w_d[n] = nc.dram_tensor(n, [128, NCH, 2 * NCH, 128], F32R,
                                kind="ExternalInput")
    b_d = {n: nc.dram_tensor(n, [E], F32, kind="ExternalInput")
           for n in ["bq", "bk", "bo", "brz", "bzz", "bgg"]}
    bv_d = nc.dram_tensor("bv", [1, E], F32, kind="ExternalInput")

    kts_d = nc.dram_tensor("kts", [B_PER_CORE, NCH, 128, L], BF16)
    outT_d = nc.dram_tensor("outT", [B_PER_CORE, E, QL], F32, kind="ExternalOutput")
    attnwT_d = nc.dram_tensor("attnwT", [B_PER_CORE, L, QL], F32,
                              kind="ExternalOutput")

    with tile.TileContext(nc) as tc, nc.allow_low_precision(
            reason="bf16 attention island feeds f32 PSUM; f32 accumulators"):
        _body(nc, tc, kinT_d, keyT_d, maskT_d, ones_d, w_d, b_d, bv_d,
              kts_d, outT_d, attnwT_d)
    nc.compile()
    return nc


def _bcast_mid(ap, n, pos):
    """Insert a step-0 dim of extent n at position pos of an AP."""
    lst = [list(p) for p in ap.ap]
    return bass.AP(tensor=ap.tensor, offset=ap.offset,
                   ap=lst[:pos] + [[0, n]] + lst[pos:])


def _body(nc, tc, kinT_d, keyT_d, maskT_d, ones_d, w_d, b_d, bv_d,
          kts_d, outT_d, attnwT_d):
    from contextlib import ExitStack
    ctx = ExitStack()
    with ctx:
        consts = ctx.enter_context(tc.tile_pool(name="consts", bufs=1))
        wpool = ctx.enter_context(tc.tile_pool(name="wpool", bufs=4))
        big = ctx.enter_context(tc.tile_pool(name="big", bufs=1))
        med = ctx.enter_context(tc.tile_pool(name="med", bufs=1))
        small = ctx.enter_context(tc.tile_pool(name="small", bufs=2))
        ps_proj = ctx.enter_context(
            tc.tile_pool(name="ps_proj", bufs=5, space="PSUM"))
        ps_half = ctx.enter_context(
            tc.tile_pool(name="ps_half", bufs=3, space="PSUM"))

        ones_t = consts.tile([128, 128], BF16, tag="ones")
        nc.sync.dma_start(ones_t[:], ones_d[:])
        bias_sb = {}
        for n, d in b_d.items():
            t = consts.tile([128, NCH], F32, tag="b_" + n)
            nc.sync.dma_start(t[:], d.rearrange("(c p) -> p c", p=128))
            bias_sb[n] = t
        bvb = consts.tile([128, E], F32, tag="bvb")
        bv_ap = bv_d[:, :]
        nc.gpsimd.dma_start(
            out=bvb[:],
            in_=bass.AP(tensor=bv_ap.tensor, offset=bv_ap.offset,
                        ap=[[0, 128], [1, E]]))

        # persistent double-buffered query staging; zero halves set once
        qzd = consts.tile([128, 2, 2, QL], BF16, tag="qzd")
        nc.gpsimd.memset(qzd[64:128, 0, 0, :], 0.0)
        nc.gpsimd.memset(qzd[0:64, 0, 1, :], 0.0)
        nc.gpsimd.memset(qzd[64:128, 1, 0, :], 0.0)
        nc.gpsimd.memset(qzd[0:64, 1, 1, :], 0.0)

        def load_wq(name, qo, dt=F32R):
            """Quarter qo (out-cols qo*256..+256) of a single [E,E] weight."""
            t = wpool.tile([128, NCH, 256], dt, tag="w", name=f"{name}_{qo}")
            nc.sync.dma_start(t[:], w_d[name][:, qo])
            return t

        def load_w8(name, mo):
            """Eighth mo (out-cols mo*128..+128) of a [2E,E] GRU cat weight."""
            t = wpool.tile([128, 2 * NCH, 128], F32R, tag="w",
                           name=f"{name}_{mo}")
            nc.sync.dma_start(t[:], w_d[name][:, mo])
            return t

        def load_oriq(b, phase):
            t = med.tile([128, NCH, QL], F32R, tag="oriq", bufs=2,
                         name=f"oriq_{phase}_{b}")
            nc.sync.dma_start(
                t[:], keyT_d[b][:, L - QL:].rearrange("(c p) t -> p c t", p=128))
            return t

        outT = [None] * B_PER_CORE
        aoutT = [None] * B_PER_CORE

        # ================= PHASE A (per batch) =================
        for b in range(B_PER_CORE):
            # weights first on the sync queue so the first matmul starts early
            wq_q = [load_wq("wq", qo, BF16) for qo in range(4)]
            kin = big.tile([128, NCH, L], BF16, tag="kin_vkm", name=f"kin{b}")
            kin_re = kinT_d[b].rearrange("(c p) t -> p c t", p=128)
            # q-projection token slice first so PE can start ASAP
            nc.sync.dma_start(kin[:, :, L - QL:], kin_re[:, :, L - QL:])
            nc.sync.dma_start(kin[:, :, 0:512], kin_re[:, :, 0:512])
            nc.sync.dma_start(kin[:, :, 512:L - QL], kin_re[:, :, 512:L - QL])
            maskt = big.tile([128, NCH, QL], BF16, tag="mask", name=f"mask{b}")
            nc.scalar.dma_start(maskt[:], maskT_d[b].rearrange("(c p) t -> p c t", p=128))
            qT = med.tile([128, NCH, QL], F32R, tag="qrh", bufs=2, name=f"qT{b}")
            for mo in range(NCH):
                p = ps_half.tile([128, QL], F32, tag="half")
                wt = wq_q[mo // 2]
                for ci in range(NCH):
                    nc.tensor.matmul(
                        p[:], wt[:, ci, (mo % 2) * 128:(mo % 2) * 128 + 128],
                        kin[:, ci, L - QL:],
                        start=(ci == 0), stop=(ci == NCH - 1))
                nc.scalar.activation(qT[:, mo, :], p[:], AF.Identity,
                                     bias=bias_sb["bq"][:, mo:mo + 1])

            # ---- k projection (streamed to DRAM scratch as bf16) ----
            wk_q = [load_wq("wk", qo, BF16) for qo in range(4)]
            for mo in range(NCH):
                wt = wk_q[mo // 2]
                for n in range(2):
                    p = ps_proj.tile([128, 512], F32, tag="proj")
                    for ci in range(NCH):
                        nc.tensor.matmul(
                            p[:], wt[:, ci, (mo % 2) * 128:(mo % 2) * 128 + 128],
                            kin[:, ci, n * 512:(n + 1) * 512],
                            start=(ci == 0), stop=(ci == NCH - 1))
                    kt = small.tile([128, 512], BF16, tag="ktmp", bufs=2)
                    nc.scalar.activation(kt[:], p[:], AF.Identity,
                                         bias=bias_sb["bk"][:, mo:mo + 1])
                    nc.gpsimd.dma_start(kts_d[b, mo, :, n * 512:(n + 1) * 512],
                                        kt[:])

            # ---- v projection (token-major, bf16); kin slot reused ----
            wv_q = [load_wq("wv", qo) for qo in range(4)]
            vkm = big.tile([128, NCH, E], BF16, tag="kin_vkm", name=f"vkm{b}")
            for kc in range(NCH):
                keyc = small.tile([128, NCH, 128], F32R, tag="keyc", bufs=1)
                nc.gpsimd.dma_start(
                    keyc[:],
                    keyT_d[b][:, kc * 128:(kc + 1) * 128]
                    .rearrange("(c p) t -> p c t", p=128))
                for q4 in range(4):
                    p = ps_half.tile([128, 256], F32, tag="half")
                    for ci in range(NCH):
                        nc.tensor.matmul(
                            p[:], keyc[:, ci, :], wv_q[q4][:, ci, :],
                            start=(ci == 0), stop=(ci == NCH - 1))
                    nc.vector.tensor_add(
                        vkm[:, kc, q4 * 256:(q4 + 1) * 256], p[:],
                        bvb[:, q4 * 256:(q4 + 1) * 256])

            # ---- attention: software-pipelined over 2-head groups ----
            awA = med.tile([128, NCH, QL], F32, tag="awA", name=f"awA{b}")
            aoutT[b] = med.tile([128, NCH, QL], F32R, tag="aoutT",
                                bufs=2, name=f"aoutT{b}")
            pTs = [None] * NCH

            def front(g):
                ktd = med.tile([128, L], BF16, tag="ktd", bufs=2,
                               name=f"ktd{b}_{g}")
                nc.scalar.dma_start(ktd[:], kts_d[b, g])
                s = g % 2
                nc.vector.tensor_copy(qzd[0:64, s, 0, :],
                                      qT[0:64, g, :].bitcast(F32))
                nc.vector.tensor_copy(qzd[64:128, s, 1, :],
                                      qT[64:128, g, :].bitcast(F32))
                pT = med.tile([128, NCH, 2, QL], BF16, tag="pT", bufs=3,
                              name=f"pT{b}_{g}")
                pTs[g] = pT
                for kc in range(NCH):
                    ps = ps_proj.tile([128, 2, QL], F32, tag="proj")
                    nc.tensor.matmul(
                        ps[:].rearrange("p a b -> p (a b)"),
                        ktd[:, kc * 128:(kc + 1) * 128],
                        qzd[:, s].rearrange("p a b -> p (a b)"),
                        start=True, stop=True)
                    nc.scalar.activation(pT[:, kc], ps[:], AF.Exp, scale=0.125)
                # coarse mask over all kc / both halves (bf16, 2x DVE)
                nc.vector.tensor_mul(pT[:], pT[:],
                                     _bcast_mid(maskt[:], 2, 2))

            def back(g):
                pT = pTs[g]
                # rowsum of masked exp, broadcast via ones-matmul
                rp = ps_proj.tile([128, 2, QL], F32, tag="proj")
                for kc in range(NCH):
                    nc.tensor.matmul(rp[:].rearrange("p a b -> p (a b)"),
                                     ones_t[:],
                                     pT[:, kc].rearrange("p a b -> p (a b)"),
                                     start=(kc == 0), stop=(kc == NCH - 1))
                r1f = small.tile([1, 2, QL], F32, tag="r1f", bufs=1)
                nc.vector.reciprocal_approx_fast(r1f[:], rp[0:1, :, :])
                r1 = small.tile([1, 2, QL], BF16, tag="r1", bufs=2)
                nc.vector.tensor_copy(r1[:], r1f[:])

                # attn @ V per head (M=64) on unnormalized masked pT
                pav = [ps_half.tile([64, QL], F32, tag="half",
                                    name=f"pav{hi}") for hi in range(2)]
                for kc in range(NCH):
                    for hi in range(2):
                        h = 2 * g + hi
                        nc.tensor.matmul(
                            pav[hi][:, :],
                            vkm[:, kc, h * 64:(h + 1) * 64],
                            pT[:, kc, hi, :],
                            start=(kc == 0), stop=(kc == NCH - 1))

                # broadcast reciprocal to all partitions
                rbp = ps_proj.tile([128, 2, QL], F32, tag="proj")
                nc.tensor.matmul(rbp[:].rearrange("p a b -> p (a b)"),
                                 ones_t[0:1, :],
                                 r1[:].rearrange("p a b -> p (a b)"),
                                 start=True, stop=True)
                rb16 = small.tile([128, 2, QL], BF16, tag="rb16", bufs=2)
                nc.scalar.activation(rb16[:], rbp[:], AF.Copy)
                recipb = small.tile([64, 2, QL], F32, tag="recipb", bufs=2)
                nc.vector.tensor_copy(recipb[:], rbp[0:64])

                # normalize attn@V during eviction; odd head partition-shifted
                nc.vector.tensor_mul(aoutT[b][0:64, g, :], pav[0][:, :],
                                     recipb[:, 0, :])
                sh = small.tile([64, QL], F32R, tag="btmp")
                nc.vector.tensor_mul(sh[:, :], pav[1][:, :],
                                     recipb[:, 1, :])
                nc.sync.dma_start(aoutT[b][64:128, g, :], sh[:, :])

                # attn_w: normalize pT in place (all-bf16 2x; attn@V already
                # consumed the unnormalized values), then f32 accumulate
                nc.vector.tensor_mul(pT[:], pT[:],
                                     _bcast_mid(rb16[:], NCH, 1))
                if g == 0:
                    nc.vector.tensor_copy(awA[:], pT[:, :, 0, :])
                else:
                    nc.vector.tensor_add(awA[:], awA[:], pT[:, :, 0, :])
                nc.vector.tensor_add(awA[:], awA[:], pT[:, :, 1, :])

            for g in range(NCH):
                front(g)
                if g > 0:
                    back(g - 1)
            back(NCH - 1)

            nc.gpsimd.tensor_scalar_mul(awA[:], awA[:], 1.0 / H)
            nc.sync.dma_start(
                attnwT_d[b].rearrange("(c p) t -> p c t", p=128), awA[:])

            # ---- out projection + relu + residual (f32r) ----
            wo_q = [load_wq("wo", qo) for qo in range(4)]
            oriq_a = load_oriq(b, "a")
            outT[b] = med.tile([128, NCH, QL], F32R, tag="outT",
                               bufs=B_PER_CORE, name=f"outT{b}")
            for mo in range(NCH):
                p = ps_half.tile([128, QL], F32, tag="half")
                wt = wo_q[mo // 2]
                for ci in range(NCH):
                    nc.tensor.matmul(
                        p[:], wt[:, ci, (mo % 2) * 128:(mo % 2) * 128 + 128],
                        aoutT[b][:, ci, :], start=(ci == 0), stop=(ci == NCH - 1))
                t = small.tile([128, QL], F32, tag="btmp")
                nc.scalar.activation(t[:], p[:], AF.Relu,
                                     bias=bias_sb["bo"][:, mo:mo + 1])
                nc.vector.tensor_add(outT[b][:, mo, :], t[:],
                                     oriq_a[:, mo, :].bitcast(F32))

        # ================= PHASE B: gated GRU (both batches) =================
        rqT = [None] * B_PER_CORE
        for b in range(B_PER_CORE):
            rqT[b] = med.tile([128, NCH, QL], F32R, tag="qrh", bufs=2, name=f"rqT{b}")
        hT = [None] * B_PER_CORE

        for stage, (wname, bias, func) in enumerate(
                [("wrC", "brz", AF.Relu), ("wgC", "bgg", AF.Tanh),
                 ("wzC", "bzz", AF.Relu)]):
            oriq_g = ([load_oriq(b, f"g{stage}") for b in range(B_PER_CORE)]
                      if stage != 1 else [None] * B_PER_CORE)
            if stage == 1:
                for b in range(B_PER_CORE):
                    hT[b] = med.tile([128, NCH, QL], F32, tag="aoutT",
                                     bufs=2, name=f"hT{b}")
            for mo in range(NCH):
                w8 = load_w8(wname, mo)
                for b in range(B_PER_CORE):
                    p = ps_half.tile([128, QL], F32, tag="half")
                    xside = oriq_g[b] if stage != 1 else rqT[b]
                    for ci in range(2 * NCH):
                        rhs = (xside[:, ci, :] if ci < NCH
                               else outT[b][:, ci - NCH, :])
                        nc.tensor.matmul(
                            p[:], w8[:, ci, :], rhs,
                            start=(ci == 0), stop=(ci == 2 * NCH - 1))
                    if stage == 0:   # r -> rq
                        t = small.tile([128, QL], F32, tag="btmp")
                        nc.scalar.activation(t[:], p[:], func,
                                             bias=bias_sb[bias][:, mo:mo + 1])
                        nc.vector.tensor_mul(rqT[b][:, mo, :], t[:],
                                             oriq_g[b][:, mo, :].bitcast(F32))
                    elif stage == 1:  # h
                        nc.scalar.activation(hT[b][:, mo, :], p[:], func,
                                             bias=bias_sb[bias][:, mo:mo + 1])
                    else:            # z + final blend + store
                        zt = small.tile([128, QL], F32, tag="btmp")
                        nc.scalar.activation(zt[:], p[:], func,
                                             bias=bias_sb[bias][:, mo:mo + 1])
                        d = small.tile([128, QL], F32, tag="btmp")
                        nc.vector.tensor_sub(d[:], hT[b][:, mo, :],
                                             oriq_g[b][:, mo, :].bitcast(F32))
                        nc.vector.tensor_mul(d[:], d[:], zt[:])
                        fin = small.tile([128, QL], F32, tag="btmp")
                        nc.vector.tensor_add(fin[:], d[:],
                                             oriq_g[b][:, mo, :].bitcast(F32))
                        nc.sync.dma_start(
                            outT_d[b][mo * 128:(mo + 1) * 128, :], fin[:])


def prep_inputs_core(core, key, pe, key_index, key_padding_mask,
                     in_proj_w, in_proj_b, out_w, out_b, gw, gb):
    b0 = core * B_PER_CORE
    sl = slice(b0, b0 + B_PER_CORE)
    import ml_dtypes as _mld
    keyc = np.asarray(key[sl], np.float32)
    kin = keyc + np.asarray(pe[sl], np.float32)
    kinT = np.ascontiguousarray(kin.transpose(0, 2, 1)).astype(_mld.bfloat16)
    keyT = np.ascontiguousarray(keyc.transpose(0, 2, 1))

    ki = np.asarray(key_index[sl])
    pad = np.asarray(key_padding_mask[sl])
    qi = ki[:, L - QL:]
    ri = ki[:, :L - QL]
    import ml_dtypes
    allowed = np.zeros((B_PER_CORE, L, QL), np.float32)
    allowed[:, :L - QL, :] = ((ri[:, :, None] < qi[:, None, :])
                              & ~pad[:, :L - QL, None])
    allowed[:, L - QL:, :] = np.eye(QL, dtype=np.float32)[None]

    w32 = lambda x: np.asarray(x, np.float32)
    im = {
        "kinT": kinT, "keyT": keyT,
        "maskT": allowed.astype(ml_dtypes.bfloat16),
        "ones": np.ones((128, 128), ml_dtypes.bfloat16),
        "bv": w32(in_proj_b[2 * E:]).reshape(1, E),
        "bq": w32(in_proj_b[:E]),
        "bk": w32(in_proj_b[E:2 * E]),
        "bo": w32(out_b),
        "brz": w32(gb["bxr"] + gb["byr"]),
        "bzz": w32(gb["bxz"] + gb["byz"]),
        "bgg": w32(gb["bxg"] + gb["byg"]),
        "wqT": _swz_q(w32(in_proj_w[:E]).T).astype(_mld.bfloat16),
        "wkT": _swz_q(w32(in_proj_w[E:2 * E]).T).astype(_mld.bfloat16),
        "wvT": _swz_q(w32(in_proj_w[2 * E:]).T),
        "woT": _swz_q(w32(out_w).T),
        "wrC": _swz_g(
            np.concatenate([w32(gw["wxr"]).T, w32(gw["wyr"]).T], 0)),
        "wzC": _swz_g(
            np.concatenate([w32(gw["wxz"]).T, w32(gw["wyz"]).T], 0)),
        "wgC": _swz_g(
            np.concatenate([w32(gw["wxg"]).T, w32(gw["wyg"]).T], 0)),
    }
    return im


def _swz_q(wT):
    """[E, E] -> [128, 4, 8, 256]: wT[c*128+p, qo*256+o] at [p, qo, c, o]."""
    return np.ascontiguousarray(
        wT.reshape(NCH, 128, 4, 256).transpose(1, 2, 0, 3))


def _swz_g(wC):
    """[2E, E] -> [128, 8, 16, 128]: wC[c*128+p, mo*128+o] at [p, mo, c, o]."""
    return np.ascontiguousarray(
        wC.reshape(2 * NCH, 128, NCH, 128).transpose(1, 2, 0, 3))


def postprocess(results):
    outs, aws = [], []
    for r in results:
        outs.append(r["outT"].transpose(0, 2, 1))
        aws.append(r["attnwT"].transpose(0, 2, 1))
    return (np.concatenate(outs, 0), np.concatenate(aws, 0))


_NC_CACHE = {}


def kernel(key, pe, key_index, key_padding_mask, query_length,
           in_proj_w, in_proj_b, out_w, out_b,
           wxr, bxr, wyr, byr, wxz, bxz, wyz, byz, wxg, bxg, wyg, byg):
    """Full-input entry point: shard B=16 across 8 NeuronCores, run, gather."""
    from concourse.bass_utils import run_bass_kernel_spmd

    key = np.asarray(key)
    assert int(query_length) == QL and key.shape == (16, L, E)
    if "nc" not in _NC_CACHE:
        _NC_CACHE["nc"] = build_kernel(num_devices=8)
    nc = _NC_CACHE["nc"]

    gw = {"wxr": wxr, "wyr": wyr, "wxz": wxz, "wyz": wyz,
          "wxg": wxg, "wyg": wyg}
    gb = {"bxr": bxr, "byr": byr, "bxz": bxz, "byz": byz,
          "bxg": bxg, "byg": byg}
    in_maps = [prep_inputs_core(c, key, pe, key_index, key_padding_mask,
                                in_proj_w, in_proj_b, out_w, out_b, gw, gb)
               for c in range(8)]
    res = run_bass_kernel_spmd(nc, in_maps, core_ids=list(range(8)))
    out, attn_w = postprocess(res.results)
    return out.astype(np.float32), attn_w.astype(np.float32)


# revision 37
# speedup vs baseline: 1.2200x; 1.0158x over previous
"""Bass/Tile kernel for EpisodeMultiheadAttentionBlock on TRN2.

Per-core: 2 batch elements (data-parallel over B=16 across 8 cores).

Layout: activations feature-major [feature parts, token free]; V token-major.
Projections/GRU matmuls float32r (1 cyc/row at free>=256); attention island
(scores, exp/pT, V, rowsum) in bf16 for 2x DVE + half DMA. Softmax without
max-subtraction; multiplicative 0/1 mask applied as ONE coarse op per head
group; 1/rowsum via ones-matmul broadcast; attn@V consumes unnormalized pT
(normalized on the [64,QL] eviction); attn_w accumulated in f32 from a
Pool-engine coarse product. Attention groups are software-pipelined
(scores/exp of group g+1 issued before the reduction half of group g).
"""

import numpy as np

import concourse.bass as bass
import concourse.mybir as mybir
import concourse.tile as tile
from concourse import bacc

F32 = mybir.dt.float32
F32R = mybir.dt.float32r
BF16 = mybir.dt.bfloat16
AF = mybir.ActivationFunctionType

B_PER_CORE = 2
L = 1024
E = 1024
H = 16
QL = 256
NCH = 8

SINGLES = ["wq", "wk", "wv", "wo"]
GRUCATS = ["wrC", "wzC", "wgC"]


def build_kernel(num_devices=8):
    nc = bacc.Bacc("TRN2", target_bir_lowering=False, debug=False,
                   num_devices=num_devices)

    kinT_d = nc.dram_tensor("kinT", [B_PER_CORE, E, L], BF16, kind="ExternalInput")
    keyT_d = nc.dram_tensor("keyT", [B_PER_CORE, E, L], F32R, kind="ExternalInput")
    maskT_d = nc.dram_tensor("maskT", [B_PER_CORE, L, QL], BF16, kind="ExternalInput")
    ones_d = nc.dram_tensor("ones", [128, 128], BF16, kind="ExternalInput")
    # weights pre-swizzled on host: [p, chunk, c, o] so DMA lines are dense
    w_d = {n: nc.dram_tensor(n + "T", [128, 4, NCH, 256],
                             BF16 if n in ("wq", "wk") else F32R,
                             kind="ExternalInput")
           for n in SINGLES}
    for n in GRUCATS:
        w_d[n] = nc.dram_tensor(n, [128, NCH, 2 * NCH, 128], F32R,
                                kind="ExternalInput")
    ball_d = nc.dram_tensor("ball", [128, 6, NCH], F32, kind="ExternalInput")
    bv_d = nc.dram_tensor("bv", [1, E], F32, kind="ExternalInput")

    kts_d = nc.dram_tensor("kts", [B_PER_CORE, NCH, 128, L], BF16)
    outT_d = nc.dram_tensor("outT", [B_PER_CORE, E, QL], F32, kind="ExternalOutput")
    attnwT_d = nc.dram_tensor("attnwT", [B_PER_CORE, L, QL], F32,
                              kind="ExternalOutput")

    with tile.TileContext(nc) as tc, nc.allow_low_precision(
            reason="bf16 attention island feeds f32 PSUM; f32 accumulators"):
        _body(nc, tc, kinT_d, keyT_d, maskT_d, ones_d, w_d, ball_d, bv_d,
              kts_d, outT_d, attnwT_d)
    nc.compile()
    return nc


def _bcast_mid(ap, n, pos):
    """Insert a step-0 dim of extent n at position pos of an AP."""
    lst = [list(p) for p in ap.ap]
    return bass.AP(tensor=ap.tensor, offset=ap.offset,
                   ap=lst[:pos] + [[0, n]] + lst[pos:])


def _body(nc, tc, kinT_d, keyT_d, maskT_d, ones_d, w_d, ball_d, bv_d,
          kts_d, outT_d, attnwT_d):
    from contextlib import ExitStack
    ctx = ExitStack()
    with ctx:
        consts = ctx.enter_context(tc.tile_pool(name="consts", bufs=1))
        wpool = ctx.enter_context(tc.tile_pool(name="wpool", bufs=4))
        big = ctx.enter_context(tc.tile_pool(name="big", bufs=1))
        med = ctx.enter_context(tc.tile_pool(name="med", bufs=1))
        small = ctx.enter_context(tc.tile_pool(name="small", bufs=2))
        ps_proj = ctx.enter_context(
            tc.tile_pool(name="ps_proj", bufs=5, space="PSUM"))
        ps_half = ctx.enter_context(
            tc.tile_pool(name="ps_half", bufs=3, space="PSUM"))

        ones_t = consts.tile([128, 128], BF16, tag="ones")
        nc.sync.dma_start(ones_t[:], ones_d[:])
        ball = consts.tile([128, 6, NCH], F32, tag="ball")
        nc.sync.dma_start(ball[:], ball_d[:])
        bias_sb = {n: ball[:, i] for i, n in
                   enumerate(["bq", "bk", "bo", "brz", "bzz", "bgg"])}
        bvb = consts.tile([128, E], F32, tag="bvb")
        bv_ap = bv_d[:, :]
        nc.gpsimd.dma_start(
            out=bvb[:],
            in_=bass.AP(tensor=bv_ap.tensor, offset=bv_ap.offset,
                        ap=[[0, 128], [1, E]]))

        # persistent double-buffered query staging; zero halves set once
        qzd = consts.tile([128, 2, 2, QL], BF16, tag="qzd")
        nc.gpsimd.memset(qzd[64:128, 0, 0, :], 0.0)
        nc.gpsimd.memset(qzd[0:64, 0, 1, :], 0.0)
        nc.gpsimd.memset(qzd[64:128, 1, 0, :], 0.0)
        nc.gpsimd.memset(qzd[0:64, 1, 1, :], 0.0)

        def load_wq(name, qo, dt=F32R):
            """Quarter qo (out-cols qo*256..+256) of a single [E,E] weight."""
            t = wpool.tile([128, NCH, 256], dt, tag="w", name=f"{name}_{qo}")
            nc.sync.dma_start(t[:], w_d[name][:, qo])
            return t

        def load_w8(name, mo):
            """Eighth mo (out-cols mo*128..+128) of a [2E,E] GRU cat weight."""
            t = wpool.tile([128, 2 * NCH, 128], F32R, tag="w",
                           name=f"{name}_{mo}")
            nc.sync.dma_start(t[:], w_d[name][:, mo])
            return t

        def load_oriq(b, phase):
            t = med.tile([128, NCH, QL], F32R, tag="oriq", bufs=2,
                         name=f"oriq_{phase}_{b}")
            nc.sync.dma_start(
                t[:], keyT_d[b][:, L - QL:].rearrange("(c p) t -> p c t", p=128))
            return t

        outT = [None] * B_PER_CORE
        aoutT = [None] * B_PER_CORE

        # ================= PHASE A (per batch) =================
        for b in range(B_PER_CORE):
            # weights first on the sync queue so the first matmul starts early
            wq_q = [load_wq("wq", qo, BF16) for qo in range(4)]
            kin = big.tile([128, NCH, L], BF16, tag="kin_vkm", name=f"kin{b}")
            kin_re = kinT_d[b].rearrange("(c p) t -> p c t", p=128)
            # q-projection token slice first so PE can start ASAP
            nc.sync.dma_start(kin[:, :, L - QL:], kin_re[:, :, L - QL:])
            nc.sync.dma_start(kin[:, :, 0:512], kin_re[:, :, 0:512])
            nc.sync.dma_start(kin[:, :, 512:L - QL], kin_re[:, :, 512:L - QL])
            maskt = big.tile([128, NCH, QL], BF16, tag="mask", name=f"mask{b}")
            nc.scalar.dma_start(maskt[:], maskT_d[b].rearrange("(c p) t -> p c t", p=128))
            qT = med.tile([128, NCH, QL], F32R, tag="qrh", bufs=2, name=f"qT{b}")
            for mo in range(NCH):
                p = ps_half.tile([128, QL], F32, tag="half")
                wt = wq_q[mo // 2]
                for ci in range(NCH):
                    nc.tensor.matmul(
                        p[:], wt[:, ci, (mo % 2) * 128:(mo % 2) * 128 + 128],
                        kin[:, ci, L - QL:],
                        start=(ci == 0), stop=(ci == NCH - 1))
                nc.scalar.activation(qT[:, mo, :], p[:], AF.Identity,
                                     bias=bias_sb["bq"][:, mo:mo + 1])

            # ---- k projection (streamed to DRAM scratch as bf16) ----
            wk_q = [load_wq("wk", qo, BF16) for qo in range(4)]
            for mo in range(NCH):
                wt = wk_q[mo // 2]
                for n in range(2):
                    p = ps_proj.tile([128, 512], F32, tag="proj")
                    for ci in range(NCH):
                        nc.tensor.matmul(
                            p[:], wt[:, ci, (mo % 2) * 128:(mo % 2) * 128 + 128],
                            kin[:, ci, n * 512:(n + 1) * 512],
                            start=(ci == 0), stop=(ci == NCH - 1))
                    kt = small.tile([128, 512], BF16, tag="ktmp", bufs=2)
                    nc.scalar.activation(kt[:], p[:], AF.Identity,
                                         bias=bias_sb["bk"][:, mo:mo + 1])
                    nc.gpsimd.dma_start(kts_d[b, mo, :, n * 512:(n + 1) * 512],
                                        kt[:])

            # ---- v projection (token-major, bf16); kin slot reused ----
            wv_q = [load_wq("wv", qo) for qo in range(4)]
            vkm = big.tile([128, NCH, E], BF16, tag="kin_vkm", name=f"vkm{b}")
            for kc in range(NCH):
                keyc = small.tile([128, NCH, 128], F32R, tag="keyc", bufs=1)
                nc.gpsimd.dma_start(
                    keyc[:],
                    keyT_d[b][:, kc * 128:(kc + 1) * 128]
                    .rearrange("(c p) t -> p c t", p=128))
                for q4 in range(4):
                    p = ps_half.tile([128, 256], F32, tag="half")
                    for ci in range(NCH):
                        nc.tensor.matmul(
                            p[:], keyc[:, ci, :], wv_q[q4][:, ci, :],
                            start=(ci == 0), stop=(ci == NCH - 1))
                    nc.vector.tensor_add(
                        vkm[:, kc, q4 * 256:(q4 + 1) * 256], p[:],
                        bvb[:, q4 * 256:(q4 + 1) * 256])

            # ---- attention: software-pipelined over 2-head groups ----
            awA = med.tile([128, NCH, QL], F32, tag="awA", name=f"awA{b}")
            aoutT[b] = med.tile([128, NCH, QL], F32R, tag="aoutT",
                                bufs=2, name=f"aoutT{b}")
            pTs = [None] * NCH

            def front(g):
                ktd = med.tile([128, L], BF16, tag="ktd", bufs=2,
                               name=f"ktd{b}_{g}")
                nc.scalar.dma_start(ktd[:], kts_d[b, g])
                s = g % 2
                nc.vector.tensor_copy(qzd[0:64, s, 0, :],
                                      qT[0:64, g, :].bitcast(F32))
                nc.vector.tensor_copy(qzd[64:128, s, 1, :],
                                      qT[64:128, g, :].bitcast(F32))
                pT = med.tile([128, NCH, 2, QL], BF16, tag="pT", bufs=3,
                              name=f"pT{b}_{g}")
                pTs[g] = pT
                for kc in range(NCH):
                    ps = ps_proj.tile([128, 2, QL], F32, tag="proj")
                    nc.tensor.matmul(
                        ps[:].rearrange("p a b -> p (a b)"),
                        ktd[:, kc * 128:(kc + 1) * 128],
                        qzd[:, s].rearrange("p a b -> p (a b)"),
                        start=True, stop=True)
                    nc.scalar.activation(pT[:, kc], ps[:], AF.Exp, scale=0.125)
                # coarse mask over all kc / both halves (bf16, 2x DVE)
                nc.vector.tensor_mul(pT[:], pT[:],
                                     _bcast_mid(maskt[:], 2, 2))

            def back(g):
                pT = pTs[g]
                # rowsum of masked exp, broadcast via ones-matmul
                rp = ps_proj.tile([128, 2, QL], F32, tag="proj")
                for kc in range(NCH):
                    nc.tensor.matmul(rp[:].rearrange("p a b -> p (a b)"),
                                     ones_t[:],
                                     pT[:, kc].rearrange("p a b -> p (a b)"),
                                     start=(kc == 0), stop=(kc == NCH - 1))
                r1f = small.tile([1, 2, QL], F32, tag="r1f", bufs=1)
                nc.vector.reciprocal_approx_fast(r1f[:], rp[0:1, :, :])
                r1 = small.tile([1, 2, QL], BF16, tag="r1", bufs=2)
                nc.vector.tensor_copy(r1[:], r1f[:])

                # attn @ V per head (M=64) on unnormalized masked pT
                pav = [ps_half.tile([64, QL], F32, tag="half",
                                    name=f"pav{hi}") for hi in range(2)]
                for kc in range(NCH):
                    for hi in range(2):
                        h = 2 * g + hi
                        nc.tensor.matmul(
                            pav[hi][:, :],
                            vkm[:, kc, h * 64:(h + 1) * 64],
                            pT[:, kc, hi, :],
                            start=(kc == 0), stop=(kc == NCH - 1))

                # broadcast reciprocal to all partitions
                rbp = ps_proj.tile([128, 2, QL], F32, tag="proj")
                nc.tensor.matmul(rbp[:].rearrange("p a b -> p (a b)"),
                                 ones_t[0:1, :],
                                 r1[:].rearrange("p a b -> p (a b)"),
                                 start=True, stop=True)
                rb16 = small.tile([128, 2, QL], BF16, tag="rb16", bufs=2)
                nc.scalar.activation(rb16[:], rbp[:], AF.Copy)
                recipb = small.tile([64, 2, QL], F32, tag="recipb", bufs=2)
                nc.vector.tensor_copy(recipb[:], rbp[0:64])

                # normalize attn@V during eviction; odd head partition-shifted
                nc.vector.tensor_mul(aoutT[b][0:64, g, :], pav[0][:, :],
                                     recipb[:, 0, :])
                sh = small.tile([64, QL], F32R, tag="btmp")
                nc.vector.tensor_mul(sh[:, :], pav[1][:, :],
                                     recipb[:, 1, :])
                nc.gpsimd.dma_start(aoutT[b][64:128, g, :], sh[:, :])

                # attn_w: normalize pT in place (all-bf16 2x; attn@V already
                # consumed the unnormalized values), then f32 accumulate
                nc.vector.tensor_mul(pT[:], pT[:],
                                     _bcast_mid(rb16[:], NCH, 1))
                if g == 0:
                    nc.vector.tensor_copy(awA[:], pT[:, :, 0, :])
                else:
                    nc.vector.tensor_add(awA[:], awA[:], pT[:, :, 0, :])
                nc.vector.tensor_add(awA[:], awA[:], pT[:, :, 1, :])

            for g in range(NCH):
                front(g)
                if g > 0:
                    back(g - 1)
            back(NCH - 1)

            nc.gpsimd.tensor_scalar_mul(awA[:], awA[:], 1.0 / H)
            nc.sync.dma_start(
                attnwT_d[b].rearrange("(c p) t -> p c t", p=128), awA[:])

            # ---- out projection + relu + residual (f32r) ----
            wo_q = [load_wq("wo", qo) for qo in range(4)]
            oriq_a = load_oriq(b, "a")
            outT[b] = med.tile([128, NCH, QL], F32R, tag="outT",
                               bufs=B_PER_CORE, name=f"outT{b}")
            for mo in range(NCH):
                p = ps_half.tile([128, QL], F32, tag="half")
                wt = wo_q[mo // 2]
                for ci in range(NCH):
                    nc.tensor.matmul(
                        p[:], wt[:, ci, (mo % 2) * 128:(mo % 2) * 128 + 128],
                        aoutT[b][:, ci, :], start=(ci == 0), stop=(ci == NCH - 1))
                t = small.tile([128, QL], F32, tag="btmp")
                nc.scalar.activation(t[:], p[:], AF.Relu,
                                     bias=bias_sb["bo"][:, mo:mo + 1])
                nc.vector.tensor_add(outT[b][:, mo, :], t[:],
                                     oriq_a[:, mo, :].bitcast(F32))

        # ================= PHASE B: gated GRU (both batches) =================
        rqT = [None] * B_PER_CORE
        for b in range(B_PER_CORE):
            rqT[b] = med.tile([128, NCH, QL], F32R, tag="qrh", bufs=2, name=f"rqT{b}")
        hT = [None] * B_PER_CORE

        for stage, (wname, bias, func) in enumerate(
                [("wrC", "brz", AF.Relu), ("wgC", "bgg", AF.Tanh),
                 ("wzC", "bzz", AF.Relu)]):
            oriq_g = ([load_oriq(b, f"g{stage}") for b in range(B_PER_CORE)]
                      if stage != 1 else [None] * B_PER_CORE)
            if stage == 1:
                for b in range(B_PER_CORE):
                    hT[b] = med.tile([128, NCH, QL], F32, tag="aoutT",
                                     bufs=2, name=f"hT{b}")
            for mo in range(NCH):
                w8 = load_w8(wname, mo)
                for b in range(B_PER_CORE):
                    p = ps_half.tile([128, QL], F32, tag="half")
                    xside = oriq_g[b] if stage != 1 else rqT[b]
                    for ci in range(2 * NCH):
                        rhs = (xside[:, ci, :] if ci < NCH
                               else outT[b][:, ci - NCH, :])
                        nc.tensor.matmul(
                            p[:], w8[:, ci, :], rhs,
                            start=(ci == 0), stop=(ci == 2 * NCH - 1))
                    if stage == 0:   # r -> rq
                        t = small.tile([128, QL], F32, tag="btmp")
                        nc.scalar.activation(t[:], p[:], func,
                                             bias=bias_sb[bias][:, mo:mo + 1])
                        nc.vector.tensor_mul(rqT[b][:, mo, :], t[:],
                                             oriq_g[b][:, mo, :].bitcast(F32))
                    elif stage == 1:  # h
                        nc.scalar.activation(hT[b][:, mo, :], p[:], func,
                                             bias=bias_sb[bias][:, mo:mo + 1])
                    else:            # z + final blend + store
                        zt = small.tile([128, QL], F32, tag="btmp")
                        nc.scalar.activation(zt[:], p[:], func,
                                             bias=bias_sb[bias][:, mo:mo + 1])
                        d = small.tile([128, QL], F32, tag="btmp")
                        nc.vector.tensor_sub(d[:], hT[b][:, mo, :],
                                             oriq_g[b][:, mo, :].bitcast(F32))
                        nc.vector.tensor_mul(d[:], d[:], zt[:])
                        fin = small.tile([128, QL], F32, tag="btmp")
                        nc.vector.tensor_add(fin[:], d[:],
                                             oriq_g[b][:, mo, :].bitcast(F32))
                        nc.sync.dma_start(
                            outT_d[b][mo * 128:(mo + 1) * 128, :], fin[:])


def prep_inputs_core(core, key, pe, key_index, key_padding_mask,
                     in_proj_w, in_proj_b, out_w, out_b, gw, gb):
    b0 = core * B_PER_CORE
    sl = slice(b0, b0 + B_PER_CORE)
    import ml_dtypes as _mld
    keyc = np.asarray(key[sl], np.float32)
    kin = keyc + np.asarray(pe[sl], np.float32)
    kinT = np.ascontiguousarray(kin.transpose(0, 2, 1)).astype(_mld.bfloat16)
    keyT = np.ascontiguousarray(keyc.transpose(0, 2, 1))

    ki = np.asarray(key_index[sl])
    pad = np.asarray(key_padding_mask[sl])
    qi = ki[:, L - QL:]
    ri = ki[:, :L - QL]
    import ml_dtypes
    allowed = np.zeros((B_PER_CORE, L, QL), np.float32)
    allowed[:, :L - QL, :] = ((ri[:, :, None] < qi[:, None, :])
                              & ~pad[:, :L - QL, None])
    allowed[:, L - QL:, :] = np.eye(QL, dtype=np.float32)[None]

    w32 = lambda x: np.asarray(x, np.float32)
    im = {
        "kinT": kinT, "keyT": keyT,
        "maskT": allowed.astype(ml_dtypes.bfloat16),
        "ones": np.ones((128, 128), ml_dtypes.bfloat16),
        "bv": w32(in_proj_b[2 * E:]).reshape(1, E),
        "ball": np.ascontiguousarray(np.stack(
            [w32(x).reshape(NCH, 128).T for x in
             (in_proj_b[:E], in_proj_b[E:2 * E], out_b,
              gb["bxr"] + gb["byr"], gb["bxz"] + gb["byz"],
              gb["bxg"] + gb["byg"])], 1)),
        "wqT": _swz_q(w32(in_proj_w[:E]).T).astype(_mld.bfloat16),
        "wkT": _swz_q(w32(in_proj_w[E:2 * E]).T).astype(_mld.bfloat16),
        "wvT": _swz_q(w32(in_proj_w[2 * E:]).T),
        "woT": _swz_q(w32(out_w).T),
        "wrC": _swz_g(
            np.concatenate([w32(gw["wxr"]).T, w32(gw["wyr"]).T], 0)),
        "wzC": _swz_g(
            np.concatenate([w32(gw["wxz"]).T, w32(gw["wyz"]).T], 0)),
        "wgC": _swz_g(
            np.concatenate([w32(gw["wxg"]).T, w32(gw["wyg"]).T], 0)),
    }
    return im


def _swz_q(wT):
    """[E, E] -> [128, 4, 8, 256]: wT[c*128+p, qo*256+o] at [p, qo, c, o]."""
    return np.ascontiguousarray(
        wT.reshape(NCH, 128, 4, 256).transpose(1, 2, 0, 3))


def _swz_g(wC):
    """[2E, E] -> [128, 8, 16, 128]: wC[c*128+p, mo*128+o] at [p, mo, c, o]."""
    return np.ascontiguousarray(
        wC.reshape(2 * NCH, 128, NCH, 128).transpose(1, 2, 0, 3))


def postprocess(results):
    outs, aws = [], []
    for r in results:
        outs.append(r["outT"].transpose(0, 2, 1))
        aws.append(r["attnwT"].transpose(0, 2, 1))
    return (np.concatenate(outs, 0), np.concatenate(aws, 0))


_NC_CACHE = {}


def kernel(key, pe, key_index, key_padding_mask, query_length,
           in_proj_w, in_proj_b, out_w, out_b,
           wxr, bxr, wyr, byr, wxz, bxz, wyz, byz, wxg, bxg, wyg, byg):
    """Full-input entry point: shard B=16 across 8 NeuronCores, run, gather."""
    from concourse.bass_utils import run_bass_kernel_spmd

    key = np.asarray(key)
    assert int(query_length) == QL and key.shape == (16, L, E)
    if "nc" not in _NC_CACHE:
        _NC_CACHE["nc"] = build_kernel(num_devices=8)
    nc = _NC_CACHE["nc"]

    gw = {"wxr": wxr, "wyr": wyr, "wxz": wxz, "wyz": wyz,
          "wxg": wxg, "wyg": wyg}
    gb = {"bxr": bxr, "byr": byr, "bxz": bxz, "byz": byz,
          "bxg": bxg, "byg": byg}
    in_maps = [prep_inputs_core(c, key, pe, key_index, key_padding_mask,
                                in_proj_w, in_proj_b, out_w, out_b, gw, gb)
               for c in range(8)]
    res = run_bass_kernel_spmd(nc, in_maps, core_ids=list(range(8)))
    out, attn_w = postprocess(res.results)
    return out.astype(np.float32), attn_w.astype(np.float32)


# revision 39
# speedup vs baseline: 1.2786x; 1.0480x over previous
"""Bass/Tile kernel for EpisodeMultiheadAttentionBlock on TRN2.

Per-core: 2 batch elements (data-parallel over B=16 across 8 cores).

Layout: activations feature-major [feature parts, token free]; V token-major.
Projections/GRU matmuls float32r (1 cyc/row at free>=256); attention island
(scores, exp/pT, V, rowsum) in bf16 for 2x DVE + half DMA. Softmax without
max-subtraction; multiplicative 0/1 mask applied as ONE coarse op per head
group; 1/rowsum via ones-matmul broadcast; attn@V consumes unnormalized pT
(normalized on the [64,QL] eviction); attn_w accumulated in f32 from a
Pool-engine coarse product. Attention groups are software-pipelined
(scores/exp of group g+1 issued before the reduction half of group g).
"""

import numpy as np

import concourse.bass as bass
import concourse.mybir as mybir
import concourse.tile as tile
from concourse import bacc

F32 = mybir.dt.float32
F32R = mybir.dt.float32r
BF16 = mybir.dt.bfloat16
AF = mybir.ActivationFunctionType

B_PER_CORE = 2
L = 1024
E = 1024
H = 16
QL = 256
NCH = 8

SINGLES = ["wq", "wk", "wv", "wo"]
GRUCATS = ["wrC", "wzC", "wgC"]


def build_kernel(num_devices=8):
    nc = bacc.Bacc("TRN2", target_bir_lowering=False, debug=False,
                   num_devices=num_devices)

    kinT_d = nc.dram_tensor("kinT", [B_PER_CORE, E, L], BF16, kind="ExternalInput")
    keyT_d = nc.dram_tensor("keyT", [B_PER_CORE, E, L], F32R, kind="ExternalInput")
    maskT_d = nc.dram_tensor("maskT", [B_PER_CORE, L, QL], BF16, kind="ExternalInput")
    ones_d = nc.dram_tensor("ones", [128, 128], BF16, kind="ExternalInput")
    # weights pre-swizzled on host: [p, chunk, c, o] so DMA lines are dense
    w_d = {n: nc.dram_tensor(n + "T", [128, 4, NCH, 256],
                             BF16 if n in ("wq", "wk") else F32R,
                             kind="ExternalInput")
           for n in SINGLES}
    for n in GRUCATS:
        w_d[n] = nc.dram_tensor(n, [128, NCH, 2 * NCH, 128], F32R,
                                kind="ExternalInput")
    ball_d = nc.dram_tensor("ball", [128, 6, NCH], F32, kind="ExternalInput")
    bv_d = nc.dram_tensor("bv", [1, E], F32, kind="ExternalInput")

    kts_d = nc.dram_tensor("kts", [B_PER_CORE, NCH, 128, L], BF16)
    outT_d = nc.dram_tensor("outT", [B_PER_CORE, E, QL], F32, kind="ExternalOutput")
    attnwT_d = nc.dram_tensor("attnwT", [B_PER_CORE, L, QL], F32,
                              kind="ExternalOutput")

    with tile.TileContext(nc) as tc, nc.allow_low_precision(
            reason="bf16 attention island feeds f32 PSUM; f32 accumulators"):
        _body(nc, tc, kinT_d, keyT_d, maskT_d, ones_d, w_d, ball_d, bv_d,
              kts_d, outT_d, attnwT_d)
    nc.compile()
    return nc


def _bcast_mid(ap, n, pos):
    """Insert a step-0 dim of extent n at position pos of an AP."""
    lst = [list(p) for p in ap.ap]
    return bass.AP(tensor=ap.tensor, offset=ap.offset,
                   ap=lst[:pos] + [[0, n]] + lst[pos:])


def _body(nc, tc, kinT_d, keyT_d, maskT_d, ones_d, w_d, ball_d, bv_d,
          kts_d, outT_d, attnwT_d):
    from contextlib import ExitStack
    ctx = ExitStack()
    with ctx:
        consts = ctx.enter_context(tc.tile_pool(name="consts", bufs=1))
        wpool = ctx.enter_context(tc.tile_pool(name="wpool", bufs=4))
        big = ctx.enter_context(tc.tile_pool(name="big", bufs=1))
        med = ctx.enter_context(tc.tile_pool(name="med", bufs=1))
        small = ctx.enter_context(tc.tile_pool(name="small", bufs=2))
        ps_proj = ctx.enter_context(
            tc.tile_pool(name="ps_proj", bufs=5, space="PSUM"))
        ps_half = ctx.enter_context(
            tc.tile_pool(name="ps_half", bufs=3, space="PSUM"))

        ones_t = consts.tile([128, 128], BF16, tag="ones")
        nc.sync.dma_start(ones_t[:], ones_d[:])
        ball = consts.tile([128, 6, NCH], F32, tag="ball")
        nc.sync.dma_start(ball[:], ball_d[:])
        bias_sb = {n: ball[:, i] for i, n in
                   enumerate(["bq", "bk", "bo", "brz", "bzz", "bgg"])}
        bvb = consts.tile([128, E], F32, tag="bvb")
        bv_ap = bv_d[:, :]
        nc.gpsimd.dma_start(
            out=bvb[:],
            in_=bass.AP(tensor=bv_ap.tensor, offset=bv_ap.offset,
                        ap=[[0, 128], [1, E]]))

        # persistent double-buffered query staging; zero halves set once
        qzd = consts.tile([128, 2, 2, QL], BF16, tag="qzd")
        nc.gpsimd.memset(qzd[64:128, 0, 0, :], 0.0)
        nc.gpsimd.memset(qzd[0:64, 0, 1, :], 0.0)
        nc.gpsimd.memset(qzd[64:128, 1, 0, :], 0.0)
        nc.gpsimd.memset(qzd[0:64, 1, 1, :], 0.0)

        def load_wq(name, qo, dt=F32R):
            """Quarter qo (out-cols qo*256..+256) of a single [E,E] weight."""
            t = wpool.tile([128, NCH, 256], dt, tag="w", name=f"{name}_{qo}")
            nc.sync.dma_start(t[:], w_d[name][:, qo])
            return t

        def load_w8(name, mo):
            """Eighth mo (out-cols mo*128..+128) of a [2E,E] GRU cat weight."""
            t = wpool.tile([128, 2 * NCH, 128], F32R, tag="w",
                           name=f"{name}_{mo}")
            nc.sync.dma_start(t[:], w_d[name][:, mo])
            return t

        def load_oriq(b, phase):
            t = med.tile([128, NCH, QL], F32R, tag="oriq", bufs=2,
                         name=f"oriq_{phase}_{b}")
            nc.sync.dma_start(
                t[:], keyT_d[b][:, L - QL:].rearrange("(c p) t -> p c t", p=128))
            return t

        outT = [None] * B_PER_CORE
        aoutT = [None] * B_PER_CORE

        # ================= PHASE A (per batch) =================
        for b in range(B_PER_CORE):
            # weights first on the sync queue so the first matmul starts early
            wq_q = [load_wq("wq", qo, BF16) for qo in range(4)]
            kin = big.tile([128, NCH, L], BF16, tag="kin_vkm", name=f"kin{b}")
            kin_re = kinT_d[b].rearrange("(c p) t -> p c t", p=128)
            # q-projection token slice first so PE can start ASAP
            nc.sync.dma_start(kin[:, :, L - QL:], kin_re[:, :, L - QL:])
            nc.sync.dma_start(kin[:, :, 0:512], kin_re[:, :, 0:512])
            nc.sync.dma_start(kin[:, :, 512:L - QL], kin_re[:, :, 512:L - QL])
            maskt = big.tile([128, NCH, QL], BF16, tag="mask", name=f"mask{b}")
            nc.scalar.dma_start(maskt[:], maskT_d[b].rearrange("(c p) t -> p c t", p=128))
            qT = med.tile([128, NCH, QL], F32R, tag="qrh", bufs=2, name=f"qT{b}")
            for mo in range(NCH):
                p = ps_half.tile([128, QL], F32, tag="half")
                wt = wq_q[mo // 2]
                for ci in range(NCH):
                    nc.tensor.matmul(
                        p[:], wt[:, ci, (mo % 2) * 128:(mo % 2) * 128 + 128],
                        kin[:, ci, L - QL:],
                        start=(ci == 0), stop=(ci == NCH - 1))
                nc.scalar.activation(qT[:, mo, :], p[:], AF.Identity,
                                     bias=bias_sb["bq"][:, mo:mo + 1])

            # ---- k projection (streamed to DRAM scratch as bf16) ----
            wk_q = [load_wq("wk", qo, BF16) for qo in range(4)]
            for mo in range(NCH):
                wt = wk_q[mo // 2]
                for n in range(2):
                    p = ps_proj.tile([128, 512], F32, tag="proj")
                    for ci in range(NCH):
                        nc.tensor.matmul(
                            p[:], wt[:, ci, (mo % 2) * 128:(mo % 2) * 128 + 128],
                            kin[:, ci, n * 512:(n + 1) * 512],
                            start=(ci == 0), stop=(ci == NCH - 1))
                    kt = small.tile([128, 512], BF16, tag="ktmp", bufs=2)
                    nc.scalar.activation(kt[:], p[:], AF.Identity,
                                         bias=bias_sb["bk"][:, mo:mo + 1])
                    nc.gpsimd.dma_start(kts_d[b, mo, :, n * 512:(n + 1) * 512],
                                        kt[:])

            # ---- v projection (token-major, bf16); kin slot reused ----
            wv_q = [load_wq("wv", qo) for qo in range(4)]
            vkm = big.tile([128, NCH, E], BF16, tag="kin_vkm", name=f"vkm{b}")
            for kc in range(NCH):
                keyc = small.tile([128, NCH, 128], F32R, tag="keyc", bufs=1)
                nc.gpsimd.dma_start(
                    keyc[:],
                    keyT_d[b][:, kc * 128:(kc + 1) * 128]
                    .rearrange("(c p) t -> p c t", p=128))
                for q4 in range(4):
                    p = ps_half.tile([128, 256], F32, tag="half")
                    for ci in range(NCH):
                        nc.tensor.matmul(
                            p[:], keyc[:, ci, :], wv_q[q4][:, ci, :],
                            start=(ci == 0), stop=(ci == NCH - 1))
                    nc.vector.tensor_add(
                        vkm[:, kc, q4 * 256:(q4 + 1) * 256], p[:],
                        bvb[:, q4 * 256:(q4 + 1) * 256])

            # ---- attention: software-pipelined over 2-head groups ----
            awA = med.tile([128, NCH, QL], F32, tag="awA", name=f"awA{b}")
            aoutT[b] = med.tile([128, NCH, QL], F32R, tag="aoutT",
                                bufs=2, name=f"aoutT{b}")
            pTs = [None] * NCH

            def front(g):
                ktd = med.tile([128, L], BF16, tag="ktd", bufs=3,
                               name=f"ktd{b}_{g}")
                nc.scalar.dma_start(ktd[:], kts_d[b, g])
                s = g % 2
                nc.vector.tensor_copy(qzd[0:64, s, 0, :],
                                      qT[0:64, g, :].bitcast(F32))
                nc.vector.tensor_copy(qzd[64:128, s, 1, :],
                                      qT[64:128, g, :].bitcast(F32))
                pT = med.tile([128, NCH, 2, QL], BF16, tag="pT", bufs=4,
                              name=f"pT{b}_{g}")
                pTs[g] = pT
                for kc in range(NCH):
                    ps = ps_proj.tile([128, 2, QL], F32, tag="proj")
                    nc.tensor.matmul(
                        ps[:].rearrange("p a b -> p (a b)"),
                        ktd[:, kc * 128:(kc + 1) * 128],
                        qzd[:, s].rearrange("p a b -> p (a b)"),
                        start=True, stop=True)
                    nc.scalar.activation(pT[:, kc], ps[:], AF.Exp, scale=0.125)
                # coarse mask over all kc / both halves (bf16, 2x DVE)
                nc.vector.tensor_mul(pT[:], pT[:],
                                     _bcast_mid(maskt[:], 2, 2))

            def back(g):
                pT = pTs[g]
                # rowsum of masked exp, broadcast via ones-matmul
                rp = ps_proj.tile([128, 2, QL], F32, tag="proj")
                for kc in range(NCH):
                    nc.tensor.matmul(rp[:].rearrange("p a b -> p (a b)"),
                                     ones_t[:],
                                     pT[:, kc].rearrange("p a b -> p (a b)"),
                                     start=(kc == 0), stop=(kc == NCH - 1))
                r1f = small.tile([1, 2, QL], F32, tag="r1f", bufs=1)
                nc.vector.reciprocal_approx_fast(r1f[:], rp[0:1, :, :])
                r1 = small.tile([1, 2, QL], BF16, tag="r1", bufs=2)
                nc.vector.tensor_copy(r1[:], r1f[:])

                # attn @ V per head (M=64) on unnormalized masked pT
                pav = [ps_half.tile([64, QL], F32, tag="half",
                                    name=f"pav{hi}") for hi in range(2)]
                for kc in range(NCH):
                    for hi in range(2):
                        h = 2 * g + hi
                        nc.tensor.matmul(
                            pav[hi][:, :],
                            vkm[:, kc, h * 64:(h + 1) * 64],
                            pT[:, kc, hi, :],
                            start=(kc == 0), stop=(kc == NCH - 1))

                # broadcast reciprocal to all partitions
                rbp = ps_proj.tile([128, 2, QL], F32, tag="proj")
                nc.tensor.matmul(rbp[:].rearrange("p a b -> p (a b)"),
                                 ones_t[0:1, :],
                                 r1[:].rearrange("p a b -> p (a b)"),
                                 start=True, stop=True)
                rb16 = small.tile([128, 2, QL], BF16, tag="rb16", bufs=2)
                nc.scalar.activation(rb16[:], rbp[:], AF.Copy)
                recipb = small.tile([64, 2, QL], F32, tag="recipb", bufs=2)
                nc.vector.tensor_copy(recipb[:], rbp[0:64])

                # normalize attn@V during eviction; odd head partition-shifted
                nc.vector.tensor_mul(aoutT[b][0:64, g, :], pav[0][:, :],
                                     recipb[:, 0, :])
                sh = small.tile([64, QL], F32R, tag="btmp")
                nc.vector.tensor_mul(sh[:, :], pav[1][:, :],
                                     recipb[:, 1, :])
                nc.gpsimd.dma_start(aoutT[b][64:128, g, :], sh[:, :])

                # attn_w: normalize pT in place (all-bf16 2x; attn@V already
                # consumed the unnormalized values), then f32 accumulate
                nc.vector.tensor_mul(pT[:], pT[:],
                                     _bcast_mid(rb16[:], NCH, 1))
                if g == 0:
                    nc.vector.tensor_copy(awA[:], pT[:, :, 0, :])
                else:
                    nc.vector.tensor_add(awA[:], awA[:], pT[:, :, 0, :])
                nc.vector.tensor_add(awA[:], awA[:], pT[:, :, 1, :])

            for g in range(NCH):
                front(g)
                if g > 0:
                    back(g - 1)
            back(NCH - 1)

            nc.gpsimd.tensor_scalar_mul(awA[:], awA[:], 1.0 / H)
            nc.sync.dma_start(
                attnwT_d[b].rearrange("(c p) t -> p c t", p=128), awA[:])

            # ---- out projection + relu + residual (f32r) ----
            wo_q = [load_wq("wo", qo) for qo in range(4)]
            oriq_a = load_oriq(b, "a")
            outT[b] = med.tile([128, NCH, QL], F32R, tag="outT",
                               bufs=B_PER_CORE, name=f"outT{b}")
            for mo in range(NCH):
                p = ps_half.tile([128, QL], F32, tag="half")
                wt = wo_q[mo // 2]
                for ci in range(NCH):
                    nc.tensor.matmul(
                        p[:], wt[:, ci, (mo % 2) * 128:(mo % 2) * 128 + 128],
                        aoutT[b][:, ci, :], start=(ci == 0), stop=(ci == NCH - 1))
                t = small.tile([128, QL], F32, tag="btmp")
                nc.scalar.activation(t[:], p[:], AF.Relu,
                                     bias=bias_sb["bo"][:, mo:mo + 1])
                nc.vector.tensor_add(outT[b][:, mo, :], t[:],
                                     oriq_a[:, mo, :].bitcast(F32))

        # ================= PHASE B: gated GRU (both batches) =================
        rqT = [None] * B_PER_CORE
        for b in range(B_PER_CORE):
            rqT[b] = med.tile([128, NCH, QL], F32R, tag="qrh", bufs=2, name=f"rqT{b}")
        hT = [None] * B_PER_CORE

        for stage, (wname, bias, func) in enumerate(
                [("wrC", "brz", AF.Relu), ("wgC", "bgg", AF.Tanh),
                 ("wzC", "bzz", AF.Relu)]):
            oriq_g = ([load_oriq(b, f"g{stage}") for b in range(B_PER_CORE)]
                      if stage != 1 else [None] * B_PER_CORE)
            if stage == 1:
                for b in range(B_PER_CORE):
                    hT[b] = med.tile([128, NCH, QL], F32, tag="aoutT",
                                     bufs=2, name=f"hT{b}")
            for mo in range(NCH):
                w8 = load_w8(wname, mo)
                for b in range(B_PER_CORE):
                    p = ps_half.tile([128, QL], F32, tag="half")
                    xside = oriq_g[b] if stage != 1 else rqT[b]
                    for ci in range(2 * NCH):
                        rhs = (xside[:, ci, :] if ci < NCH
                               else outT[b][:, ci - NCH, :])
                        nc.tensor.matmul(
                            p[:], w8[:, ci, :], rhs,
                            start=(ci == 0), stop=(ci == 2 * NCH - 1))
                    if stage == 0:   # r -> rq
                        t = small.tile([128, QL], F32, tag="btmp")
                        nc.scalar.activation(t[:], p[:], func,
                                             bias=bias_sb[bias][:, mo:mo + 1])
                        nc.vector.tensor_mul(rqT[b][:, mo, :], t[:],
                                             oriq_g[b][:, mo, :].bitcast(F32))
                    elif stage == 1:  # h
                        nc.scalar.activation(hT[b][:, mo, :], p[:], func,
                                             bias=bias_sb[bias][:, mo:mo + 1])
                    else:            # z + final blend + store
                        zt = small.tile([128, QL], F32, tag="btmp")
                        nc.scalar.activation(zt[:], p[:], func,
                                             bias=bias_sb[bias][:, mo:mo + 1])
                        d = small.tile([128, QL], F32, tag="btmp")
                        nc.vector.tensor_sub(d[:], hT[b][:, mo, :],
                                             oriq_g[b][:, mo, :].bitcast(F32))
                        nc.vector.tensor_mul(d[:], d[:], zt[:])
                        fin = small.tile([128, QL], F32, tag="btmp")
                        nc.vector.tensor_add(fin[:], d[:],
                                             oriq_g[b][:, mo, :].bitcast(F32))
                        nc.sync.dma_start(
                            outT_d[b][mo * 128:(mo + 1) * 128, :], fin[:])


def prep_inputs_core(core, key, pe, key_index, key_padding_mask,
                     in_proj_w, in_proj_b, out_w, out_b, gw, gb):
    b0 = core * B_PER_CORE
    sl = slice(b0, b0 + B_PER_CORE)
    import ml_dtypes as _mld
    keyc = np.asarray(key[sl], np.float32)
    kin = keyc + np.asarray(pe[sl], np.float32)
    kinT = np.ascontiguousarray(kin.transpose(0, 2, 1)).astype(_mld.bfloat16)
    keyT = np.ascontiguousarray(keyc.transpose(0, 2, 1))

    ki = np.asarray(key_index[sl])
    pad = np.asarray(key_padding_mask[sl])
    qi = ki[:, L - QL:]
    ri = ki[:, :L - QL]
    import ml_dtypes
    allowed = np.zeros((B_PER_CORE, L, QL), np.float32)
    allowed[:, :L - QL, :] = ((ri[:, :, None] < qi[:, None, :])
                              & ~pad[:, :L - QL, None])
    allowed[:, L - QL:, :] = np.eye(QL, dtype=np.float32)[None]

    w32 = lambda x: np.asarray(x, np.float32)
    im = {
        "kinT": kinT, "keyT": keyT,
        "maskT": allowed.astype(ml_dtypes.bfloat16),
        "ones": np.ones((128, 128), ml_dtypes.bfloat16),
        "bv": w32(in_proj_b[2 * E:]).reshape(1, E),
        "ball": np.ascontiguousarray(np.stack(
            [w32(x).reshape(NCH, 128).T for x in
             (in_proj_b[:E], in_proj_b[E:2 * E], out_b,
              gb["bxr"] + gb["byr"], gb["bxz"] + gb["byz"],
              gb["bxg"] + gb["byg"])], 1)),
        "wqT": _swz_q(w32(in_proj_w[:E]).T).astype(_mld.bfloat16),
        "wkT": _swz_q(w32(in_proj_w[E:2 * E]).T).astype(_mld.bfloat16),
        "wvT": _swz_q(w32(in_proj_w[2 * E:]).T),
        "woT": _swz_q(w32(out_w).T),
        "wrC": _swz_g(
            np.concatenate([w32(gw["wxr"]).T, w32(gw["wyr"]).T], 0)),
        "wzC": _swz_g(
            np.concatenate([w32(gw["wxz"]).T, w32(gw["wyz"]).T], 0)),
        "wgC": _swz_g(
            np.concatenate([w32(gw["wxg"]).T, w32(gw["wyg"]).T], 0)),
    }
    return im


def _swz_q(wT):
    """[E, E] -> [128, 4, 8, 256]: wT[c*128+p, qo*256+o] at [p, qo, c, o]."""
    return np.ascontiguousarray(
        wT.reshape(NCH, 128, 4, 256).transpose(1, 2, 0, 3))


def _swz_g(wC):
    """[2E, E] -> [128, 8, 16, 128]: wC[c*128+p, mo*128+o] at [p, mo, c, o]."""
    return np.ascontiguousarray(
        wC.reshape(2 * NCH, 128, NCH, 128).transpose(1, 2, 0, 3))


def postprocess(results):
    outs, aws = [], []
    for r in results:
        outs.append(r["outT"].transpose(0, 2, 1))
        aws.append(r["attnwT"].transpose(0, 2, 1))
    return (np.concatenate(outs, 0), np.concatenate(aws, 0))


_NC_CACHE = {}


def kernel(key, pe, key_index, key_padding_mask, query_length,
           in_proj_w, in_proj_b, out_w, out_b,
           wxr, bxr, wyr, byr, wxz, bxz, wyz, byz, wxg, bxg, wyg, byg):
    """Full-input entry point: shard B=16 across 8 NeuronCores, run, gather."""
    from concourse.bass_utils import run_bass_kernel_spmd

    key = np.asarray(key)
    assert int(query_length) == QL and key.shape == (16, L, E)
    if "nc" not in _NC_CACHE:
        _NC_CACHE["nc"] = build_kernel(num_devices=8)
    nc = _NC_CACHE["nc"]

    gw = {"wxr": wxr, "wyr": wyr, "wxz": wxz, "wyz": wyz,
          "wxg": wxg, "wyg": wyg}
    gb = {"bxr": bxr, "byr": byr, "bxz": bxz, "byz": byz,
          "bxg": bxg, "byg": byg}
    in_maps = [prep_inputs_core(c, key, pe, key_index, key_padding_mask,
                                in_proj_w, in_proj_b, out_w, out_b, gw, gb)
               for c in range(8)]
    res = run_bass_kernel_spmd(nc, in_maps, core_ids=list(range(8)))
    out, attn_w = postprocess(res.results)
    return out.astype(np.float32), attn_w.astype(np.float32)
